# revision 5
# baseline (speedup 1.0000x reference)
"""Trainium2 Bass kernel for nn_DimReceiver (moe_routing), pure data-parallel
over 8 NeuronCores.

Math (per row):
  x1         = relu([ctx, msg0] @ W1 + b1)                  # [520] -> [64]
  dim_logits = x1 @ Wc + bc                                 # [8]
  dim        = argmax(dim_logits + G0)   (G0 = Gumbel noise of key(42), host-made)
  dim_probs  = softmax(dim_logits)
  the_dim    = ctx[dim*64 : dim*64+64]                      # per-row gather
  x2         = relu([the_dim, msg1] @ Wt1 + bt1)
  x3         = relu(x2 @ Wt2 + bt2)
  t_logits   = x3 @ Wt + bt                                 # [64]
  target     = argmax(t_logits + G1)     (G1 = Gumbel noise of key(43))
  target_probs = softmax(t_logits)

Device layout notes:
  - Matmuls run feature-major ("transposed": features on partitions, rows on the
    free axis) so the contraction dim always sits on the partition axis. The big
    input `contexts` is host-pre-transposed (ctxt [512, BC]) so no on-chip
    transpose of it is needed.
  - the_dim is fetched with a dma_gather straight from HBM using on-device
    computed indices (row*8 + dim). The gather's HW layout imposes a fixed row
    permutation sigma within each 2048-row block; the target branch runs in
    sigma order and the host un-permutes its outputs.
  - Gumbel noise is precomputed on host with jax (CPU) so sampling matches
    jax.random.categorical bit-for-bit up to fp32 matmul rounding.
"""

import sys

sys.path.insert(0, "/opt/trn_rl_repo")

import numpy as np

import concourse.bacc as bacc
import concourse.bass as bass
from concourse import masks, mybir, tile
from concourse.bass_utils import run_bass_kernel_spmd

F32 = mybir.dt.float32
I32 = mybir.dt.int32
I16 = mybir.dt.int16
AF = mybir.ActivationFunctionType
ALU = mybir.AluOpType
AX = mybir.AxisListType

B = 262144
CTX = 512
NDIMS = 8
TGT = 64
DS = 64  # DIM_SIZE
N_CORES = 8

# per-2048 gather row permutation: device position i holds source row perm[i]
PERM2048 = (np.arange(2048) % 16) * 128 + (np.arange(2048) // 16)


def build_program(BC, n_cores=N_CORES, debug=False, enable_asserts=False, variant="full"):
    """Build + compile the Bass/Tile program for BC rows per core.

    variant: "full" | "nogather" (memset the_dim) | "constgather" (iota idxs)
    """
    assert BC % 2048 == 0
    NG2 = BC // 2048  # gather groups
    nc = bacc.Bacc(
        "TRN2",
        target_bir_lowering=False,
        debug=debug,
        enable_asserts=enable_asserts,
        num_devices=n_cores,
    )

    ctx_d = nc.dram_tensor("ctx", [BC, CTX], F32, kind="ExternalInput").ap()
    ctxt_d = nc.dram_tensor("ctxt", [CTX, BC], F32, kind="ExternalInput").ap()
    msg0t_d = nc.dram_tensor("msg0t", [8, BC], F32, kind="ExternalInput").ap()
    msg1st_d = nc.dram_tensor("msg1st", [2, BC], F32, kind="ExternalInput").ap()
    aux_d = nc.dram_tensor("aux", [BC // 512, 128, 288], F32, kind="ExternalInput").ap()
    w1a_d = nc.dram_tensor("w1a", [128, 256], F32, kind="ExternalInput").ap()
    w1b_d = nc.dram_tensor("w1b", [8, 64], F32, kind="ExternalInput").ap()
    wc_d = nc.dram_tensor("wc", [64, 8], F32, kind="ExternalInput").ap()
    wt1a_d = nc.dram_tensor("wt1a", [64, 64], F32, kind="ExternalInput").ap()
    wt1b_d = nc.dram_tensor("wt1b", [2, 64], F32, kind="ExternalInput").ap()
    wt2_d = nc.dram_tensor("wt2", [64, 32], F32, kind="ExternalInput").ap()
    wt_d = nc.dram_tensor("wt", [32, 64], F32, kind="ExternalInput").ap()
    b1c_d = nc.dram_tensor("b1c", [64, 1], F32, kind="ExternalInput").ap()
    bt1c_d = nc.dram_tensor("bt1c", [64, 1], F32, kind="ExternalInput").ap()
    bt2c_d = nc.dram_tensor("bt2c", [32, 1], F32, kind="ExternalInput").ap()
    btc_d = nc.dram_tensor("btc", [64, 1], F32, kind="ExternalInput").ap()
    bct_d = nc.dram_tensor("bctile", [128, 32], F32, kind="ExternalInput").ap()
    out_d = nc.dram_tensor("out", [NG2, 128, 1184], F32, kind="ExternalOutput").ap()

    # [512, BC] -> [p, k, g, r] with row = k*128+p, col = g*512+r
    ctxt_v = ctxt_d.rearrange("(k p) (g r) -> p k g r", p=128, r=512)

    with tile.TileContext(nc) as tc:
        with (
            tc.tile_pool(name="const", bufs=1) as cpool,
            tc.tile_pool(name="sb", bufs=3) as sb,
            tc.tile_pool(name="aux", bufs=8) as auxp,
            tc.tile_pool(name="stage", bufs=2) as stp,
            tc.tile_pool(name="gath", bufs=2) as gap,
            tc.tile_pool(name="msg", bufs=2) as msgp,
            tc.tile_pool(name="psx1", bufs=2, space="PSUM") as psx,
            tc.tile_pool(name="psch", bufs=6, space="PSUM") as psc,
        ):
            # ---- constants ----
            id128 = cpool.tile([128, 128], F32, tag="id128")
            masks.make_identity(nc, id128[:])
            repl = cpool.tile([16, 128], F32, tag="repl")
            nc.gpsimd.memset(repl[:], 0.0)
            # repl[k, 16a+m] = (k == m): iota (p - m0) over [[0,8],[-1,16]]
            nc.gpsimd.affine_select(
                out=repl[:], in_=repl[:], compare_op=ALU.not_equal, fill=1.0,
                base=0, pattern=[[0, 8], [-1, 16]], channel_multiplier=1,
            )
            ir64i = cpool.tile([128, 256], I32, tag="ir64i")
            nc.gpsimd.iota(ir64i[:], pattern=[[0, 4], [-1, 64]], base=64, channel_multiplier=0)
            ir64f = cpool.tile([128, 256], F32, tag="ir64f")
            nc.vector.tensor_copy(ir64f[:], ir64i[:])
            ir8i = cpool.tile([128, 32], I32, tag="ir8i")
            nc.gpsimd.iota(ir8i[:], pattern=[[0, 4], [-1, 8]], base=8, channel_multiplier=0)
            ir8f = cpool.tile([128, 32], F32, tag="ir8f")
            nc.vector.tensor_copy(ir8f[:], ir8i[:])
            i8ji = cpool.tile([16, 128], I32, tag="i8ji")
            nc.gpsimd.iota(i8ji[:], pattern=[[8, 128]], base=0, channel_multiplier=0)
            i8jf = cpool.tile([16, 128], F32, tag="i8jf")
            nc.vector.tensor_copy(i8jf[:], i8ji[:])
            qveci = cpool.tile([16, 1], I32, tag="qveci")
            nc.gpsimd.iota(qveci[:], pattern=[[0, 1]], base=0, channel_multiplier=1024)
            qvecf = cpool.tile([16, 1], F32, tag="qvecf")
            nc.vector.tensor_copy(qvecf[:], qveci[:])

            def cload(name, shape, src):
                t = cpool.tile(shape, F32, tag=name)
                nc.sync.dma_start(out=t[:], in_=src)
                return t

            w1a = cload("w1a", [128, 4, 64], w1a_d)
            w1b = cload("w1b", [8, 64], w1b_d)
            wc = cload("wc", [64, 8], wc_d)
            wt1a = cload("wt1a", [64, 64], wt1a_d)
            wt1b = cload("wt1b", [2, 64], wt1b_d)
            wt2 = cload("wt2", [64, 32], wt2_d)
            wt = cload("wt", [32, 64], wt_d)
            b1c = cload("b1c", [64, 1], b1c_d)
            bt1c = cload("bt1c", [64, 1], bt1c_d)
            bt2c = cload("bt2c", [32, 1], bt2c_d)
            btc = cload("btc", [64, 1], btc_d)
            bct = cload("bct", [128, 32], bct_d)

            for gg in range(NG2):
                msg0t = msgp.tile([8, 2048], F32, tag="msg0t")
                nc.gpsimd.dma_start(out=msg0t[:], in_=msg0t_d[:, gg * 2048:(gg + 1) * 2048])
                msg1st = msgp.tile([2, 2048], F32, tag="msg1st")
                nc.gpsimd.dma_start(out=msg1st[:], in_=msg1st_d[:, gg * 2048:(gg + 1) * 2048])
                dim_f = gap.tile([128, 16], F32, tag="dim_f")
                ostage = stp.tile([128, 1184], F32, tag="ostage")
                auxs = []

                # ---------- dim branch (natural row order) ----------
                for g4 in range(4):
                    g = gg * 4 + g4
                    ctxt_sb = sb.tile([128, 4, 512], F32, tag="ctxt_sb")
                    nc.sync.dma_start(out=ctxt_sb[:], in_=ctxt_v[:, :, g, :])
                    aux_sb = auxp.tile([128, 288], F32, tag="aux_sb")
                    nc.scalar.dma_start(out=aux_sb[:], in_=aux_d[g])
                    auxs.append(aux_sb)

                    x1t_ps = psx.tile([64, 512], F32, tag="x1t")
                    for k in range(4):
                        nc.tensor.matmul(
                            x1t_ps[:], w1a[:, k, :], ctxt_sb[:, k, :],
                            start=(k == 0), stop=False,
                        )
                    nc.tensor.matmul(
                        x1t_ps[:], w1b[:], msg0t[:, g4 * 512:(g4 + 1) * 512],
                        start=False, stop=True,
                    )
                    x1t = sb.tile([64, 512], F32, tag="x1t_sb")
                    nc.scalar.activation(x1t[:], x1t_ps[:], AF.Relu, bias=b1c[:])

                    dl_ps = psc.tile([128, 32], F32, tag="chain")
                    for c in range(4):
                        nc.tensor.matmul(
                            dl_ps[:, c * 8:(c + 1) * 8],
                            x1t[:, c * 128:(c + 1) * 128], wc[:],
                            start=True, stop=True,
                        )
                    zb = sb.tile([128, 32], F32, tag="zb")
                    nc.vector.tensor_tensor(zb[:], dl_ps[:], bct[:], ALU.add)
                    e8 = sb.tile([128, 32], F32, tag="e8")
                    nc.scalar.activation(e8[:], zb[:], AF.Exp)
                    z2 = sb.tile([128, 32], F32, tag="z2")
                    nc.vector.tensor_tensor(z2[:], zb[:], aux_sb[:, 0:32], ALU.add)
                    z2v = z2[:].rearrange("p (g j) -> p g j", j=8)
                    m2 = sb.tile([128, 4], F32, tag="m2")
                    nc.vector.tensor_reduce(m2[:], z2v, axis=AX.X, op=ALU.max)
                    m2b = m2[:].unsqueeze(-1).broadcast_to([128, 4, 8])
                    eq = sb.tile([128, 32], F32, tag="eq8")
                    nc.vector.tensor_tensor(eq[:].rearrange("p (g j) -> p g j", j=8), z2v, m2b, ALU.is_equal)
                    t8 = sb.tile([128, 32], F32, tag="t8")
                    nc.vector.tensor_tensor(t8[:], eq[:], ir8f[:], ALU.mult)
                    rm = sb.tile([128, 4], F32, tag="rm")
                    nc.vector.tensor_reduce(rm[:], t8[:].rearrange("p (g j) -> p g j", j=8), axis=AX.X, op=ALU.max)
                    nc.vector.tensor_scalar(
                        dim_f[:, g4 * 4:(g4 + 1) * 4], rm[:], 8.0, -1.0, ALU.subtract, ALU.mult
                    )
                    s8 = sb.tile([128, 4], F32, tag="s8")
                    nc.vector.tensor_reduce(s8[:], e8[:].rearrange("p (g j) -> p g j", j=8), axis=AX.X, op=ALU.add)
                    r8 = sb.tile([128, 4], F32, tag="r8")
                    nc.vector.reciprocal(r8[:], s8[:])
                    r8b = r8[:].unsqueeze(-1).broadcast_to([128, 4, 8])
                    nc.vector.tensor_tensor(
                        ostage[:, g4 * 32:(g4 + 1) * 32].rearrange("p (g j) -> p g j", j=8),
                        e8[:].rearrange("p (g j) -> p g j", j=8), r8b, ALU.mult,
                    )

                # ---------- gather indices + dma_gather ----------
                dimT_ps = psc.tile([16, 128], F32, tag="chain")
                nc.tensor.transpose(dimT_ps[:], dim_f[:], id128[:])
                idxf = sb.tile([16, 128], F32, tag="idxf")
                nc.scalar.activation(idxf[:], dimT_ps[:], AF.Identity, bias=qvecf[:])
                idxf2 = sb.tile([16, 128], F32, tag="idxf2")
                nc.vector.tensor_tensor(idxf2[:], idxf[:], i8jf[:], ALU.add)
                idxr_ps = psc.tile([128, 128], F32, tag="chain")
                nc.tensor.matmul(idxr_ps[:], repl[:], idxf2[:], start=True, stop=True)
                idxs16 = sb.tile([128, 128], I16, tag="idxs16")
                if variant == "constgather":
                    nc.gpsimd.iota(idxs16[:], pattern=[[8, 128]], base=0, channel_multiplier=0)
                else:
                    nc.scalar.copy(idxs16[:], idxr_ps[:])
                gdim = gap.tile([128, 16, 64], F32, tag="gdim")
                table = ctx_d[gg * 2048:(gg + 1) * 2048, :].rearrange("r (d e) -> (r d) e", e=64)
                if variant == "nogather":
                    nc.gpsimd.memset(gdim[:], 0.125)
                else:
                    # one dma_gather is capped at ~1024 idxs (128 SWDGE ring
                    # descriptors per Q7 core); two half-gathers preserve the
                    # exact same position->row mapping
                    for h in range(2):
                        nc.gpsimd.dma_gather(
                            out_ap=gdim[:, h * 8:(h + 1) * 8, :], in_ap=table,
                            idxs_ap=idxs16[:, h * 64:(h + 1) * 64],
                            num_idxs=1024, num_idxs_reg=1024, elem_size=64,
                        )
                # dim output (int32 bits into the f32 staging tile)
                nc.vector.tensor_copy(ostage[:, 1152:1168].bitcast(I32), dim_f[:])

                # ---------- target branch (sigma row order) ----------
                for g4 in range(4):
                    aux_sb = auxs[g4]
                    tdT_ps = psc.tile([64, 512], F32, tag="chain")
                    for c in range(4):
                        nc.tensor.transpose(
                            tdT_ps[:, c * 128:(c + 1) * 128], gdim[:, g4 * 4 + c, :], id128[:]
                        )
                    tdT = sb.tile([64, 512], F32, tag="tdT")
                    nc.scalar.copy(tdT[:], tdT_ps[:])
                    x2t_ps = psc.tile([64, 512], F32, tag="chain")
                    nc.tensor.matmul(x2t_ps[:], wt1a[:], tdT[:], start=True, stop=False)
                    nc.tensor.matmul(
                        x2t_ps[:], wt1b[:], msg1st[:, g4 * 512:(g4 + 1) * 512],
                        start=False, stop=True,
                    )
                    x2t = sb.tile([64, 512], F32, tag="x2t")
                    nc.scalar.activation(x2t[:], x2t_ps[:], AF.Relu, bias=bt1c[:])
                    x3t_ps = psc.tile([32, 512], F32, tag="chain")
                    nc.tensor.matmul(x3t_ps[:], wt2[:], x2t[:], start=True, stop=True)
                    x3t = sb.tile([32, 512], F32, tag="x3t")
                    nc.scalar.activation(x3t[:], x3t_ps[:], AF.Relu, bias=bt2c[:])
                    tlt_ps = psc.tile([64, 512], F32, tag="chain")
                    nc.tensor.matmul(tlt_ps[:], wt[:], x3t[:], start=True, stop=True)
                    tlt = sb.tile([64, 512], F32, tag="tlt")
                    nc.scalar.activation(tlt[:], tlt_ps[:], AF.Identity, bias=btc[:])
                    tl_ps = psc.tile([128, 256], F32, tag="chain")
                    for c in range(4):
                        nc.tensor.transpose(
                            tl_ps[:, c * 64:(c + 1) * 64],
                            tlt[:, c * 128:(c + 1) * 128], id128[:64, :64],
                        )
                    e64 = sb.tile([128, 256], F32, tag="e64")
                    nc.scalar.activation(e64[:], tl_ps[:], AF.Exp)
                    z2t = sb.tile([128, 256], F32, tag="z2t")
                    nc.vector.tensor_tensor(z2t[:], tl_ps[:], aux_sb[:, 32:288], ALU.add)
                    z2tv = z2t[:].rearrange("p (g j) -> p g j", j=64)
                    m2t = sb.tile([128, 4], F32, tag="m2t")
                    nc.vector.tensor_reduce(m2t[:], z2tv, axis=AX.X, op=ALU.max)
                    m2tb = m2t[:].unsqueeze(-1).broadcast_to([128, 4, 64])
                    eqt = sb.tile([128, 256], F32, tag="eqt")
                    nc.vector.tensor_tensor(eqt[:].rearrange("p (g j) -> p g j", j=64), z2tv, m2tb, ALU.is_equal)
                    t64 = sb.tile([128, 256], F32, tag="t64")
                    nc.vector.tensor_tensor(t64[:], eqt[:], ir64f[:], ALU.mult)
                    rmt = sb.tile([128, 4], F32, tag="rmt")
                    nc.vector.tensor_reduce(rmt[:], t64[:].rearrange("p (g j) -> p g j", j=64), axis=AX.X, op=ALU.max)
                    nc.vector.tensor_scalar(
                        ostage[:, 1168 + g4 * 4:1168 + (g4 + 1) * 4].bitcast(I32),
                        rmt[:], 64.0, -1.0, ALU.subtract, ALU.mult,
                    )
                    s64 = sb.tile([128, 4], F32, tag="s64")
                    nc.vector.tensor_reduce(s64[:], e64[:].rearrange("p (g j) -> p g j", j=64), axis=AX.X, op=ALU.add)
                    r64 = sb.tile([128, 4], F32, tag="r64")
                    nc.vector.reciprocal(r64[:], s64[:])
                    r64b = r64[:].unsqueeze(-1).broadcast_to([128, 4, 64])
                    nc.vector.tensor_tensor(
                        ostage[:, 128 + g4 * 256:128 + (g4 + 1) * 256].rearrange("p (g j) -> p g j", j=64),
                        e64[:].rearrange("p (g j) -> p g j", j=64), r64b, ALU.mult,
                    )

                nc.sync.dma_start(out=out_d[gg], in_=ostage[:])

    nc.compile()
    return nc


def host_prepare(inputs, BC, n_cores=N_CORES):
    """Slice/transform full inputs into per-core in_maps."""
    import jax
    import jax.numpy as jnp

    contexts = np.ascontiguousarray(np.asarray(inputs["contexts"], dtype=np.float32))
    msg0 = np.asarray(inputs["msg0"], dtype=np.float32)
    msg1 = np.asarray(inputs["msg1"], dtype=np.float32)
    W1 = np.asarray(inputs["W1"], dtype=np.float32)
    b1 = np.asarray(inputs["b1"], dtype=np.float32)
    Wc = np.asarray(inputs["Wc"], dtype=np.float32)
    bc = np.asarray(inputs["bc"], dtype=np.float32)
    Wt1 = np.asarray(inputs["Wt1"], dtype=np.float32)
    bt1 = np.asarray(inputs["bt1"], dtype=np.float32)
    Wt2 = np.asarray(inputs["Wt2"], dtype=np.float32)
    bt2 = np.asarray(inputs["bt2"], dtype=np.float32)
    Wt = np.asarray(inputs["Wt"], dtype=np.float32)
    bt = np.asarray(inputs["bt"], dtype=np.float32)

    Btot = contexts.shape[0]
    assert Btot == BC * n_cores

    cpu = jax.devices("cpu")[0]
    with jax.default_device(cpu):
        G0 = np.asarray(jax.random.gumbel(jax.random.key(42), (Btot, NDIMS), jnp.float32))
        G1 = np.asarray(jax.random.gumbel(jax.random.key(43), (Btot, TGT), jnp.float32))

    NG2 = BC // 2048
    NG = BC // 512

    shared = {
        "w1a": np.ascontiguousarray(
            W1[:512].reshape(4, 128, 64).transpose(1, 0, 2).reshape(128, 256)
        ),
        "w1b": np.ascontiguousarray(W1[512:520]),
        "wc": Wc,
        "wt1a": np.ascontiguousarray(Wt1[:64]),
        "wt1b": np.ascontiguousarray(Wt1[64:66]),
        "wt2": Wt2,
        "wt": Wt,
        "b1c": b1[:, None],
        "bt1c": bt1[:, None],
        "bt2c": bt2[:, None],
        "btc": bt[:, None],
        "bctile": np.ascontiguousarray(np.tile(bc, (128, 4))),
    }

    in_maps = []
    for c in range(n_cores):
        sl = slice(c * BC, (c + 1) * BC)
        ctx_c = contexts[sl]
        ctxt_c = np.ascontiguousarray(ctx_c.T)
        msg0t_c = np.ascontiguousarray(msg0[sl].T)
        msg1s = msg1[sl].reshape(NG2, 2048, 2)[:, PERM2048, :].reshape(BC, 2)
        msg1st_c = np.ascontiguousarray(msg1s.T)
        g0t = np.ascontiguousarray(
            G0[sl].reshape(NG, 4, 128, 8).transpose(0, 2, 1, 3).reshape(NG, 128, 32)
        )
        g1s = G1[sl].reshape(NG2, 2048, 64)[:, PERM2048, :]
        g1t = g1s.reshape(NG2, 4, 4, 128, 64).transpose(0, 1, 3, 2, 4).reshape(NG, 128, 256)
        aux_c = np.ascontiguousarray(np.concatenate([g0t, g1t], axis=2))
        in_maps.append(
            dict(
                ctx=ctx_c, ctxt=ctxt_c, msg0t=msg0t_c, msg1st=msg1st_c, aux=aux_c,
                **shared,
            )
        )
    return in_maps


def host_unpack(results, BC, n_cores=N_CORES):
    """Reassemble per-core 'out' arrays into the full reference output tuple."""
    NG2 = BC // 2048
    Btot = BC * n_cores
    dim_probs = np.empty((Btot, NDIMS), np.float32)
    target_probs = np.empty((Btot, TGT), np.float32)
    dim = np.empty((Btot,), np.int32)
    target = np.empty((Btot,), np.int32)
    for c in range(n_cores):
        o = results[c]["out"]  # [NG2, 128, 1184] f32
        sl = slice(c * BC, (c + 1) * BC)
        dp = o[:, :, 0:128].reshape(NG2, 128, 4, 4, 8).transpose(0, 2, 3, 1, 4)
        dim_probs[sl] = dp.reshape(BC, NDIMS)
        tp_dev = o[:, :, 128:1152].reshape(NG2, 128, 4, 4, 64).transpose(0, 2, 3, 1, 4)
        tp_dev = tp_dev.reshape(NG2, 2048, 64)
        tp = np.empty_like(tp_dev)
        tp[:, PERM2048, :] = tp_dev
        target_probs[sl] = tp.reshape(BC, TGT)
        dim_dev = np.ascontiguousarray(o[:, :, 1152:1168]).view(np.int32)
        dim[sl] = dim_dev.transpose(0, 2, 1).reshape(BC)
        tg_dev = np.ascontiguousarray(o[:, :, 1168:1184]).view(np.int32)
        tg_dev = tg_dev.transpose(0, 2, 1).reshape(NG2, 2048)
        tg = np.empty_like(tg_dev)
        tg[:, PERM2048] = tg_dev
        target[sl] = tg.reshape(BC)
    return dim_probs, target_probs, dim, target


_CACHE = {}


def _get_program(BC, n_cores):
    key = (BC, n_cores)
    if key not in _CACHE:
        _CACHE[key] = build_program(BC, n_cores)
    return _CACHE[key]


def run(inputs, BC=B // N_CORES, n_cores=N_CORES, trace=False, **kw):
    nc = _get_program(BC, n_cores)
    in_maps = host_prepare(inputs, BC, n_cores)
    res = run_bass_kernel_spmd(nc, in_maps, core_ids=list(range(n_cores)), trace=trace, **kw)
    return host_unpack(res.results, BC, n_cores), res


def kernel(**inputs):
    (dim_probs, target_probs, dim, target), _ = run(inputs)
    return dim_probs, target_probs, dim, target


# revision 7
# speedup vs baseline: 1.2703x; 1.2703x over previous
"""Trainium2 Bass kernel for nn_DimReceiver (moe_routing), pure data-parallel
over 8 NeuronCores.

Math (per row):
  x1         = relu([ctx, msg0] @ W1 + b1)                  # [520] -> [64]
  dim_logits = x1 @ Wc + bc                                 # [8]
  dim        = argmax(dim_logits + G0)   (G0 = Gumbel noise of key(42), host-made)
  dim_probs  = softmax(dim_logits)
  the_dim    = ctx[dim*64 : dim*64+64]                      # per-row gather
  x2         = relu([the_dim, msg1] @ Wt1 + bt1)
  x3         = relu(x2 @ Wt2 + bt2)
  t_logits   = x3 @ Wt + bt                                 # [64]
  target     = argmax(t_logits + G1)     (G1 = Gumbel noise of key(43))
  target_probs = softmax(t_logits)

Device layout / numerics notes:
  - Matmuls run feature-major (features on partitions, rows on the free axis).
    fp32 matmuls on TRN2 run 5x slower (fp32_mode=LOW_HIGH), so the big
    contractions use exact bf16 hi+lo splits prepared on host:
      x @ W = hi@Whi + hi@Wlo + lo@Whi  (+ lo@Wlo ~ 2^-18, dropped)
    which is fp32-accurate at bf16 matmul speed.
  - the_dim is fetched with dma_gather(transpose=True) from a host-built
    interleaved table ctx_hl[row, dim] = [c0hi, c0lo, c1hi, c1lo, ...] so the
    gathered tile lands feature-major with hi/lo pairs on partitions; weights
    with duplicated rows (W'[2c]=W'[2c+1]=W[c]) consume it directly:
      x2 = gdimT@Whi_dup + gdimT@Wlo_dup  (exact product of (hi+lo)(Whi+Wlo))
  - One dma_gather is capped at ~1024 idxs (128 SWDGE ring descriptors per Q7
    core); two half-gathers per 2048-row block preserve the position mapping.
  - The gather's HW layout imposes a fixed row permutation sigma within each
    2048-row block; the target branch runs in sigma order and the host
    un-permutes its outputs.
  - Gumbel noise is precomputed on host with jax (CPU) so sampling matches
    jax.random.categorical bit-for-bit up to matmul rounding.
"""

import sys

sys.path.insert(0, "/opt/trn_rl_repo")

import ml_dtypes
import numpy as np

import concourse.bacc as bacc
import concourse.bass as bass
from concourse import masks, mybir, tile
from concourse.bass_utils import run_bass_kernel_spmd

F32 = mybir.dt.float32
BF16 = mybir.dt.bfloat16
I32 = mybir.dt.int32
I16 = mybir.dt.int16
AF = mybir.ActivationFunctionType
ALU = mybir.AluOpType
AX = mybir.AxisListType
BF = np.dtype(ml_dtypes.bfloat16)

B = 262144
CTX = 512
NDIMS = 8
TGT = 64
N_CORES = 8

# per-2048 gather row permutation: device position i holds source row perm[i]
PERM2048 = (np.arange(2048) % 16) * 128 + (np.arange(2048) // 16)


def build_program(BC, n_cores=N_CORES, debug=False, enable_asserts=False, variant="full"):
    """Build + compile the Bass/Tile program for BC rows per core."""
    assert BC % 2048 == 0
    NG2 = BC // 2048  # gather groups
    nc = bacc.Bacc(
        "TRN2",
        target_bir_lowering=False,
        debug=debug,
        enable_asserts=enable_asserts,
        num_devices=n_cores,
    )

    # gather table: [BC, 1024] bf16, row r = [c0hi c0lo c1hi c1lo ...]
    ctxhl_d = nc.dram_tensor("ctxhl", [BC, 1024], BF16, kind="ExternalInput").ap()
    # x1 streams: rows 0..511 = ctxT_hi k-tiles, 512..1023 = ctxT_lo
    ctxt2_d = nc.dram_tensor("ctxt2", [1024, BC], BF16, kind="ExternalInput").ap()
    msg0hl_d = nc.dram_tensor("msg0hl", [16, BC], BF16, kind="ExternalInput").ap()
    msg1hl_d = nc.dram_tensor("msg1hl", [4, BC], BF16, kind="ExternalInput").ap()
    aux_d = nc.dram_tensor("aux", [BC // 512, 128, 288], F32, kind="ExternalInput").ap()
    w1ah_d = nc.dram_tensor("w1ah", [128, 256], BF16, kind="ExternalInput").ap()
    w1al_d = nc.dram_tensor("w1al", [128, 256], BF16, kind="ExternalInput").ap()
    w1bh_d = nc.dram_tensor("w1bh", [16, 64], BF16, kind="ExternalInput").ap()
    w1bl_d = nc.dram_tensor("w1bl", [16, 64], BF16, kind="ExternalInput").ap()
    wc_d = nc.dram_tensor("wc", [64, 8], F32, kind="ExternalInput").ap()
    wt1ah_d = nc.dram_tensor("wt1ah", [128, 64], BF16, kind="ExternalInput").ap()
    wt1al_d = nc.dram_tensor("wt1al", [128, 64], BF16, kind="ExternalInput").ap()
    wt1bh_d = nc.dram_tensor("wt1bh", [4, 64], BF16, kind="ExternalInput").ap()
    wt1bl_d = nc.dram_tensor("wt1bl", [4, 64], BF16, kind="ExternalInput").ap()
    wt2_d = nc.dram_tensor("wt2", [64, 32], F32, kind="ExternalInput").ap()
    wt_d = nc.dram_tensor("wt", [32, 64], F32, kind="ExternalInput").ap()
    b1c_d = nc.dram_tensor("b1c", [64, 1], F32, kind="ExternalInput").ap()
    bt1c_d = nc.dram_tensor("bt1c", [64, 1], F32, kind="ExternalInput").ap()
    bt2c_d = nc.dram_tensor("bt2c", [32, 1], F32, kind="ExternalInput").ap()
    bct_d = nc.dram_tensor("bctile", [128, 32], F32, kind="ExternalInput").ap()
    btt_d = nc.dram_tensor("bttile", [128, 256], F32, kind="ExternalInput").ap()
    out_d = nc.dram_tensor("out", [NG2, 128, 1184], F32, kind="ExternalOutput").ap()

    # [1024, BC] -> [p, k, g, r] with row = k*128+p, col = g*512+r
    ctxt2_v = ctxt2_d.rearrange("(k p) (g r) -> p k g r", p=128, r=512)

    with tile.TileContext(nc) as tc:
        with (
            tc.tile_pool(name="const", bufs=1) as cpool,
            tc.tile_pool(name="sb", bufs=3) as sb,
            tc.tile_pool(name="aux", bufs=8) as auxp,
            tc.tile_pool(name="stage", bufs=2) as stp,
            tc.tile_pool(name="gath", bufs=2) as gap,
            tc.tile_pool(name="msg", bufs=2) as msgp,
            tc.tile_pool(name="psx1", bufs=3, space="PSUM") as psx,
            tc.tile_pool(name="psch", bufs=5, space="PSUM") as psc,
        ):
            # ---- constants ----
            id128 = cpool.tile([128, 128], F32, tag="id128")
            masks.make_identity(nc, id128[:])
            repl = cpool.tile([16, 128], F32, tag="repl")
            nc.gpsimd.memset(repl[:], 0.0)
            nc.gpsimd.affine_select(
                out=repl[:], in_=repl[:], compare_op=ALU.not_equal, fill=1.0,
                base=0, pattern=[[0, 8], [-1, 16]], channel_multiplier=1,
            )
            ir64i = cpool.tile([128, 256], I32, tag="ir64i")
            nc.gpsimd.iota(ir64i[:], pattern=[[0, 4], [-1, 64]], base=64, channel_multiplier=0)
            ir64f = cpool.tile([128, 256], F32, tag="ir64f")
            nc.vector.tensor_copy(ir64f[:], ir64i[:])
            ir8i = cpool.tile([128, 32], I32, tag="ir8i")
            nc.gpsimd.iota(ir8i[:], pattern=[[0, 4], [-1, 8]], base=8, channel_multiplier=0)
            ir8f = cpool.tile([128, 32], F32, tag="ir8f")
            nc.vector.tensor_copy(ir8f[:], ir8i[:])
            i8ji = cpool.tile([16, 128], I32, tag="i8ji")
            nc.gpsimd.iota(i8ji[:], pattern=[[8, 128]], base=0, channel_multiplier=0)
            i8jf = cpool.tile([16, 128], F32, tag="i8jf")
            nc.vector.tensor_copy(i8jf[:], i8ji[:])
            qveci = cpool.tile([16, 1], I32, tag="qveci")
            nc.gpsimd.iota(qveci[:], pattern=[[0, 1]], base=0, channel_multiplier=1024)
            qvecf = cpool.tile([16, 1], F32, tag="qvecf")
            nc.vector.tensor_copy(qvecf[:], qveci[:])

            def cload(name, shape, src, dt=F32):
                t = cpool.tile(shape, dt, tag=name)
                nc.sync.dma_start(out=t[:], in_=src)
                return t

            w1ah = cload("w1ah", [128, 4, 64], w1ah_d, BF16)
            w1al = cload("w1al", [128, 4, 64], w1al_d, BF16)
            w1bh = cload("w1bh", [16, 64], w1bh_d, BF16)
            w1bl = cload("w1bl", [16, 64], w1bl_d, BF16)
            wc = cload("wc", [64, 8], wc_d)
            wt1ah = cload("wt1ah", [128, 64], wt1ah_d, BF16)
            wt1al = cload("wt1al", [128, 64], wt1al_d, BF16)
            wt1bh = cload("wt1bh", [4, 64], wt1bh_d, BF16)
            wt1bl = cload("wt1bl", [4, 64], wt1bl_d, BF16)
            wt2 = cload("wt2", [64, 32], wt2_d)
            wt = cload("wt", [32, 64], wt_d)
            b1c = cload("b1c", [64, 1], b1c_d)
            bt1c = cload("bt1c", [64, 1], bt1c_d)
            bt2c = cload("bt2c", [32, 1], bt2c_d)
            bct = cload("bct", [128, 32], bct_d)
            btt = cload("btt", [128, 256], btt_d)

            for gg in range(NG2):
                msg0hl = msgp.tile([16, 2048], BF16, tag="msg0hl")
                nc.gpsimd.dma_start(out=msg0hl[:], in_=msg0hl_d[:, gg * 2048:(gg + 1) * 2048])
                msg1hl = msgp.tile([4, 2048], BF16, tag="msg1hl")
                nc.gpsimd.dma_start(out=msg1hl[:], in_=msg1hl_d[:, gg * 2048:(gg + 1) * 2048])
                dim_f = gap.tile([128, 16], F32, tag="dim_f")
                ostage = stp.tile([128, 1184], F32, tag="ostage")
                auxs = []

                # ---------- dim branch (natural row order) ----------
                for g4 in range(4):
                    g = gg * 4 + g4
                    ctxt_sb = sb.tile([128, 8, 512], BF16, tag="ctxt_sb")
                    nc.sync.dma_start(out=ctxt_sb[:], in_=ctxt2_v[:, :, g, :])
                    aux_sb = auxp.tile([128, 288], F32, tag="aux_sb")
                    nc.scalar.dma_start(out=aux_sb[:], in_=aux_d[g])
                    auxs.append(aux_sb)

                    # x1^T = sum_k (hi@Whi + hi@Wlo + lo@Whi) + msg0 pairs
                    x1t_ps = psx.tile([64, 512], F32, tag="x1t")
                    for k in range(4):
                        nc.tensor.matmul(x1t_ps[:], w1ah[:, k, :], ctxt_sb[:, k, :],
                                         start=(k == 0), stop=False)
                        nc.tensor.matmul(x1t_ps[:], w1al[:, k, :], ctxt_sb[:, k, :],
                                         start=False, stop=False)
                        nc.tensor.matmul(x1t_ps[:], w1ah[:, k, :], ctxt_sb[:, 4 + k, :],
                                         start=False, stop=False)
                    m0 = msg0hl[:, g4 * 512:(g4 + 1) * 512]
                    nc.tensor.matmul(x1t_ps[:], w1bh[:], m0, start=False, stop=False)
                    nc.tensor.matmul(x1t_ps[:], w1bl[:], m0, start=False, stop=True)
                    x1t = sb.tile([64, 512], F32, tag="x1t_sb")
                    nc.scalar.activation(x1t[:], x1t_ps[:], AF.Relu, bias=b1c[:])

                    dl_ps = psc.tile([128, 32], F32, tag="chain")
                    for c in range(4):
                        nc.tensor.matmul(
                            dl_ps[:, c * 8:(c + 1) * 8],
                            x1t[:, c * 128:(c + 1) * 128], wc[:],
                            start=True, stop=True,
                        )
                    zb = sb.tile([128, 32], F32, tag="zb")
                    nc.vector.tensor_tensor(zb[:], dl_ps[:], bct[:], ALU.add)
                    e8 = sb.tile([128, 32], F32, tag="e8")
                    nc.scalar.activation(e8[:], zb[:], AF.Exp)
                    z2 = sb.tile([128, 32], F32, tag="z2")
                    nc.vector.tensor_tensor(z2[:], zb[:], aux_sb[:, 0:32], ALU.add)
                    z2v = z2[:].rearrange("p (g j) -> p g j", j=8)
                    m2 = sb.tile([128, 4], F32, tag="m2")
                    nc.vector.tensor_reduce(m2[:], z2v, axis=AX.X, op=ALU.max)
                    m2b = m2[:].unsqueeze(-1).broadcast_to([128, 4, 8])
                    eq = sb.tile([128, 32], F32, tag="eq8")
                    nc.vector.tensor_tensor(eq[:].rearrange("p (g j) -> p g j", j=8), z2v, m2b, ALU.is_equal)
                    t8 = sb.tile([128, 32], F32, tag="t8")
                    nc.vector.tensor_tensor(t8[:], eq[:], ir8f[:], ALU.mult)
                    rm = sb.tile([128, 4], F32, tag="rm")
                    nc.vector.tensor_reduce(rm[:], t8[:].rearrange("p (g j) -> p g j", j=8), axis=AX.X, op=ALU.max)
                    nc.vector.tensor_scalar(
                        dim_f[:, g4 * 4:(g4 + 1) * 4], rm[:], 8.0, -1.0, ALU.subtract, ALU.mult
                    )
                    s8 = sb.tile([128, 4], F32, tag="s8")
                    nc.vector.tensor_reduce(s8[:], e8[:].rearrange("p (g j) -> p g j", j=8), axis=AX.X, op=ALU.add)
                    r8 = sb.tile([128, 4], F32, tag="r8")
                    nc.vector.reciprocal(r8[:], s8[:])
                    r8b = r8[:].unsqueeze(-1).broadcast_to([128, 4, 8])
                    nc.vector.tensor_tensor(
                        ostage[:, g4 * 32:(g4 + 1) * 32].rearrange("p (g j) -> p g j", j=8),
                        e8[:].rearrange("p (g j) -> p g j", j=8), r8b, ALU.mult,
                    )

                # ---------- gather indices + dma_gather (transposed) ----------
                dimT_ps = psc.tile([16, 128], F32, tag="chain")
                nc.tensor.transpose(dimT_ps[:], dim_f[:], id128[:])
                idxf = sb.tile([16, 128], F32, tag="idxf")
                nc.scalar.activation(idxf[:], dimT_ps[:], AF.Identity, bias=qvecf[:])
                idxf2 = sb.tile([16, 128], F32, tag="idxf2")
                nc.vector.tensor_tensor(idxf2[:], idxf[:], i8jf[:], ALU.add)
                idxr_ps = psc.tile([128, 128], F32, tag="chain")
                nc.tensor.matmul(idxr_ps[:], repl[:], idxf2[:], start=True, stop=True)
                idxs16 = sb.tile([128, 128], I16, tag="idxs16")
                nc.scalar.copy(idxs16[:], idxr_ps[:])
                # gathered, feature-major with hi/lo pairs on partitions:
                # gdimT[2c+h, D] = {hi,lo}(the_dim[perm row of D, c])
                gdimT = gap.tile([128, 2048], BF16, tag="gdimT")
                table = ctxhl_d[gg * 2048:(gg + 1) * 2048, :].rearrange("r (d e) -> (r d) e", e=128)
                if variant == "nogather":
                    nc.gpsimd.memset(gdimT[:], 0.125)
                else:
                    for h in range(2):
                        nc.gpsimd.dma_gather(
                            out_ap=gdimT[:, h * 1024:(h + 1) * 1024].rearrange("p (o n) -> p o n", o=1),
                            in_ap=table,
                            idxs_ap=idxs16[:, h * 64:(h + 1) * 64],
                            num_idxs=1024, num_idxs_reg=1024, elem_size=128,
                            transpose=True, single_packet=False,
                        )
                # dim output (int32 bits into the f32 staging tile)
                nc.vector.tensor_copy(ostage[:, 1152:1168].bitcast(I32), dim_f[:])

                # ---------- target branch (sigma row order) ----------
                for g4 in range(4):
                    aux_sb = auxs[g4]
                    gd = gdimT[:, g4 * 512:(g4 + 1) * 512]
                    x2t_ps = psc.tile([64, 512], F32, tag="chain")
                    nc.tensor.matmul(x2t_ps[:], wt1ah[:], gd, start=True, stop=False)
                    nc.tensor.matmul(x2t_ps[:], wt1al[:], gd, start=False, stop=False)
                    m1 = msg1hl[:, g4 * 512:(g4 + 1) * 512]
                    nc.tensor.matmul(x2t_ps[:], wt1bh[:], m1, start=False, stop=False)
                    nc.tensor.matmul(x2t_ps[:], wt1bl[:], m1, start=False, stop=True)
                    x2t = sb.tile([64, 512], F32, tag="x2t")
                    nc.scalar.activation(x2t[:], x2t_ps[:], AF.Relu, bias=bt1c[:])
                    x3t_ps = psc.tile([32, 512], F32, tag="chain")
                    nc.tensor.matmul(x3t_ps[:], wt2[:], x2t[:], start=True, stop=True)
                    x3t = sb.tile([32, 512], F32, tag="x3t")
                    nc.scalar.activation(x3t[:], x3t_ps[:], AF.Relu, bias=bt2c[:])
                    # target logits straight in natural orientation: per chunk
                    # tl[128, 64] = x3t_chunk.T @ Wt
                    tl_ps = psc.tile([128, 256], F32, tag="chain")
                    for c in range(4):
                        nc.tensor.matmul(
                            tl_ps[:, c * 64:(c + 1) * 64],
                            x3t[:, c * 128:(c + 1) * 128], wt[:],
                            start=True, stop=True,
                        )
                    tlb = sb.tile([128, 256], F32, tag="tlb")
                    nc.vector.tensor_tensor(tlb[:], tl_ps[:], btt[:], ALU.add)
                    e64 = sb.tile([128, 256], F32, tag="e64")
                    nc.scalar.activation(e64[:], tlb[:], AF.Exp)
                    z2t = sb.tile([128, 256], F32, tag="z2t")
                    nc.vector.tensor_tensor(z2t[:], tlb[:], aux_sb[:, 32:288], ALU.add)
                    z2tv = z2t[:].rearrange("p (g j) -> p g j", j=64)
                    m2t = sb.tile([128, 4], F32, tag="m2t")
                    nc.vector.tensor_reduce(m2t[:], z2tv, axis=AX.X, op=ALU.max)
                    m2tb = m2t[:].unsqueeze(-1).broadcast_to([128, 4, 64])
                    eqt = sb.tile([128, 256], F32, tag="eqt")
                    nc.vector.tensor_tensor(eqt[:].rearrange("p (g j) -> p g j", j=64), z2tv, m2tb, ALU.is_equal)
                    t64 = sb.tile([128, 256], F32, tag="t64")
                    nc.vector.tensor_tensor(t64[:], eqt[:], ir64f[:], ALU.mult)
                    rmt = sb.tile([128, 4], F32, tag="rmt")
                    nc.vector.tensor_reduce(rmt[:], t64[:].rearrange("p (g j) -> p g j", j=64), axis=AX.X, op=ALU.max)
                    nc.vector.tensor_scalar(
                        ostage[:, 1168 + g4 * 4:1168 + (g4 + 1) * 4].bitcast(I32),
                        rmt[:], 64.0, -1.0, ALU.subtract, ALU.mult,
                    )
                    s64 = sb.tile([128, 4], F32, tag="s64")
                    nc.vector.tensor_reduce(s64[:], e64[:].rearrange("p (g j) -> p g j", j=64), axis=AX.X, op=ALU.add)
                    r64 = sb.tile([128, 4], F32, tag="r64")
                    nc.vector.reciprocal(r64[:], s64[:])
                    r64b = r64[:].unsqueeze(-1).broadcast_to([128, 4, 64])
                    nc.vector.tensor_tensor(
                        ostage[:, 128 + g4 * 256:128 + (g4 + 1) * 256].rearrange("p (g j) -> p g j", j=64),
                        e64[:].rearrange("p (g j) -> p g j", j=64), r64b, ALU.mult,
                    )

                nc.sync.dma_start(out=out_d[gg], in_=ostage[:])

    nc.compile()
    return nc


def _split_hl(x):
    """Exact bf16 hi+lo split of an f32 array."""
    hi = x.astype(BF)
    lo = (x - hi.astype(np.float32)).astype(BF)
    return hi, lo


def _interleave_cols(hi, lo):
    """[..., n] x2 -> [..., 2n] with [..., 2c]=hi, [..., 2c+1]=lo."""
    out = np.empty(hi.shape[:-1] + (hi.shape[-1] * 2,), dtype=hi.dtype)
    out[..., 0::2] = hi
    out[..., 1::2] = lo
    return out


def _dup_rows(w):
    """[k, m] -> [2k, m] with rows duplicated pairwise."""
    return np.repeat(w, 2, axis=0)


def host_prepare(inputs, BC, n_cores=N_CORES):
    """Slice/transform full inputs into per-core in_maps."""
    import jax
    import jax.numpy as jnp

    contexts = np.ascontiguousarray(np.asarray(inputs["contexts"], dtype=np.float32))
    msg0 = np.asarray(inputs["msg0"], dtype=np.float32)
    msg1 = np.asarray(inputs["msg1"], dtype=np.float32)
    W1 = np.asarray(inputs["W1"], dtype=np.float32)
    b1 = np.asarray(inputs["b1"], dtype=np.float32)
    Wc = np.asarray(inputs["Wc"], dtype=np.float32)
    bc = np.asarray(inputs["bc"], dtype=np.float32)
    Wt1 = np.asarray(inputs["Wt1"], dtype=np.float32)
    bt1 = np.asarray(inputs["bt1"], dtype=np.float32)
    Wt2 = np.asarray(inputs["Wt2"], dtype=np.float32)
    bt2 = np.asarray(inputs["bt2"], dtype=np.float32)
    Wt = np.asarray(inputs["Wt"], dtype=np.float32)
    bt = np.asarray(inputs["bt"], dtype=np.float32)

    Btot = contexts.shape[0]
    assert Btot == BC * n_cores

    cpu = jax.devices("cpu")[0]
    with jax.default_device(cpu):
        G0 = np.asarray(jax.random.gumbel(jax.random.key(42), (Btot, NDIMS), jnp.float32))
        G1 = np.asarray(jax.random.gumbel(jax.random.key(43), (Btot, TGT), jnp.float32))

    NG2 = BC // 2048
    NG = BC // 512

    # weight preps (shared across cores)
    W1a_hi, W1a_lo = _split_hl(W1[:512])
    w1ah = np.ascontiguousarray(W1a_hi.reshape(4, 128, 64).transpose(1, 0, 2).reshape(128, 256))
    w1al = np.ascontiguousarray(W1a_lo.reshape(4, 128, 64).transpose(1, 0, 2).reshape(128, 256))
    W1b_hi, W1b_lo = _split_hl(W1[512:520])
    Wt1a_hi, Wt1a_lo = _split_hl(Wt1[:64])
    Wt1b_hi, Wt1b_lo = _split_hl(Wt1[64:66])
    shared = {
        "w1ah": w1ah, "w1al": w1al,
        "w1bh": _dup_rows(W1b_hi), "w1bl": _dup_rows(W1b_lo),
        "wc": Wc,
        "wt1ah": _dup_rows(Wt1a_hi), "wt1al": _dup_rows(Wt1a_lo),
        "wt1bh": _dup_rows(Wt1b_hi), "wt1bl": _dup_rows(Wt1b_lo),
        "wt2": Wt2, "wt": Wt,
        "b1c": b1[:, None], "bt1c": bt1[:, None], "bt2c": bt2[:, None],
        "bctile": np.ascontiguousarray(np.tile(bc, (128, 4))),
        "bttile": np.ascontiguousarray(np.tile(bt, (128, 4))),
    }

    in_maps = []
    for c in range(n_cores):
        sl = slice(c * BC, (c + 1) * BC)
        ctx_c = contexts[sl]
        hi, lo = _split_hl(ctx_c)
        ctxhl_c = np.ascontiguousarray(_interleave_cols(hi, lo))  # [BC, 1024]
        ctxt2_c = np.ascontiguousarray(
            np.concatenate([hi.T, lo.T], axis=0)
        )  # [1024, BC]
        m0h, m0l = _split_hl(msg0[sl])
        msg0hl_c = np.ascontiguousarray(_interleave_cols(m0h, m0l).T)  # [16, BC]
        msg1s = msg1[sl].reshape(NG2, 2048, 2)[:, PERM2048, :].reshape(BC, 2)
        m1h, m1l = _split_hl(msg1s)
        msg1hl_c = np.ascontiguousarray(_interleave_cols(m1h, m1l).T)  # [4, BC]
        g0t = np.ascontiguousarray(
            G0[sl].reshape(NG, 4, 128, 8).transpose(0, 2, 1, 3).reshape(NG, 128, 32)
        )
        g1s = G1[sl].reshape(NG2, 2048, 64)[:, PERM2048, :]
        g1t = g1s.reshape(NG2, 4, 4, 128, 64).transpose(0, 1, 3, 2, 4).reshape(NG, 128, 256)
        aux_c = np.ascontiguousarray(np.concatenate([g0t, g1t], axis=2))
        in_maps.append(
            dict(
                ctxhl=ctxhl_c, ctxt2=ctxt2_c, msg0hl=msg0hl_c, msg1hl=msg1hl_c,
                aux=aux_c, **shared,
            )
        )
    return in_maps


def host_unpack(results, BC, n_cores=N_CORES):
    """Reassemble per-core 'out' arrays into the full reference output tuple."""
    NG2 = BC // 2048
    Btot = BC * n_cores
    dim_probs = np.empty((Btot, NDIMS), np.float32)
    target_probs = np.empty((Btot, TGT), np.float32)
    dim = np.empty((Btot,), np.int32)
    target = np.empty((Btot,), np.int32)
    for c in range(n_cores):
        o = results[c]["out"]  # [NG2, 128, 1184] f32
        sl = slice(c * BC, (c + 1) * BC)
        dp = o[:, :, 0:128].reshape(NG2, 128, 4, 4, 8).transpose(0, 2, 3, 1, 4)
        dim_probs[sl] = dp.reshape(BC, NDIMS)
        tp_dev = o[:, :, 128:1152].reshape(NG2, 128, 4, 4, 64).transpose(0, 2, 3, 1, 4)
        tp_dev = tp_dev.reshape(NG2, 2048, 64)
        tp = np.empty_like(tp_dev)
        tp[:, PERM2048, :] = tp_dev
        target_probs[sl] = tp.reshape(BC, TGT)
        dim_dev = np.ascontiguousarray(o[:, :, 1152:1168]).view(np.int32)
        dim[sl] = dim_dev.transpose(0, 2, 1).reshape(BC)
        tg_dev = np.ascontiguousarray(o[:, :, 1168:1184]).view(np.int32)
        tg_dev = tg_dev.transpose(0, 2, 1).reshape(NG2, 2048)
        tg = np.empty_like(tg_dev)
        tg[:, PERM2048] = tg_dev
        target[sl] = tg.reshape(BC)
    return dim_probs, target_probs, dim, target


_CACHE = {}


def _get_program(BC, n_cores):
    key = (BC, n_cores)
    if key not in _CACHE:
        _CACHE[key] = build_program(BC, n_cores)
    return _CACHE[key]


def run(inputs, BC=B // N_CORES, n_cores=N_CORES, trace=False, **kw):
    nc = _get_program(BC, n_cores)
    in_maps = host_prepare(inputs, BC, n_cores)
    res = run_bass_kernel_spmd(nc, in_maps, core_ids=list(range(n_cores)), trace=trace, **kw)
    return host_unpack(res.results, BC, n_cores), res


def kernel(**inputs):
    (dim_probs, target_probs, dim, target), _ = run(inputs)
    return dim_probs, target_probs, dim, target


# revision 8
# speedup vs baseline: 1.3013x; 1.0244x over previous
"""Trainium2 Bass kernel for nn_DimReceiver (moe_routing), pure data-parallel
over 8 NeuronCores.

Math (per row):
  x1         = relu([ctx, msg0] @ W1 + b1)                  # [520] -> [64]
  dim_logits = x1 @ Wc + bc                                 # [8]
  dim        = argmax(dim_logits + G0)   (G0 = Gumbel noise of key(42), host-made)
  dim_probs  = softmax(dim_logits)
  the_dim    = ctx[dim*64 : dim*64+64]                      # per-row gather
  x2         = relu([the_dim, msg1] @ Wt1 + bt1)
  x3         = relu(x2 @ Wt2 + bt2)
  t_logits   = x3 @ Wt + bt                                 # [64]
  target     = argmax(t_logits + G1)     (G1 = Gumbel noise of key(43))
  target_probs = softmax(t_logits)

Device layout / numerics notes:
  - Matmuls run feature-major (features on partitions, rows on the free axis).
    fp32 matmuls on TRN2 run 5x slower (fp32_mode=LOW_HIGH), so the big
    contractions use exact bf16 hi+lo splits prepared on host:
      x @ W = hi@Whi + hi@Wlo + lo@Whi  (+ lo@Wlo ~ 2^-18, dropped)
    which is fp32-accurate at bf16 matmul speed.
  - the_dim is fetched with dma_gather(transpose=True) from a host-built
    interleaved table ctx_hl[row, dim] = [c0hi, c0lo, c1hi, c1lo, ...] so the
    gathered tile lands feature-major with hi/lo pairs on partitions; weights
    with duplicated rows (W'[2c]=W'[2c+1]=W[c]) consume it directly:
      x2 = gdimT@Whi_dup + gdimT@Wlo_dup  (exact product of (hi+lo)(Whi+Wlo))
  - One dma_gather is capped at ~1024 idxs (128 SWDGE ring descriptors per Q7
    core); two half-gathers per 2048-row block preserve the position mapping.
  - The gather's HW layout imposes a fixed row permutation sigma within each
    2048-row block; the target branch runs in sigma order and the host
    un-permutes its outputs.
  - Gumbel noise is precomputed on host with jax (CPU) so sampling matches
    jax.random.categorical bit-for-bit up to matmul rounding.
"""

import sys

sys.path.insert(0, "/opt/trn_rl_repo")

import ml_dtypes
import numpy as np

import concourse.bacc as bacc
import concourse.bass as bass
from concourse import masks, mybir, tile
from concourse.bass_utils import run_bass_kernel_spmd

F32 = mybir.dt.float32
BF16 = mybir.dt.bfloat16
I32 = mybir.dt.int32
I16 = mybir.dt.int16
AF = mybir.ActivationFunctionType
ALU = mybir.AluOpType
AX = mybir.AxisListType
BF = np.dtype(ml_dtypes.bfloat16)

B = 262144
CTX = 512
NDIMS = 8
TGT = 64
N_CORES = 8

# per-2048 gather row permutation: device position i holds source row perm[i]
PERM2048 = (np.arange(2048) % 16) * 128 + (np.arange(2048) // 16)


def build_program(BC, n_cores=N_CORES, debug=False, enable_asserts=False, variant="full"):
    """Build + compile the Bass/Tile program for BC rows per core."""
    assert BC % 2048 == 0
    NG2 = BC // 2048  # gather groups
    nc = bacc.Bacc(
        "TRN2",
        target_bir_lowering=False,
        debug=debug,
        enable_asserts=enable_asserts,
        num_devices=n_cores,
    )

    # gather table: [BC, 1024] bf16, row r = [c0hi c0lo c1hi c1lo ...]
    ctxhl_d = nc.dram_tensor("ctxhl", [BC, 1024], BF16, kind="ExternalInput").ap()
    # x1 streams: rows 0..511 = ctxT_hi k-tiles, 512..1023 = ctxT_lo
    ctxt2_d = nc.dram_tensor("ctxt2", [1024, BC], BF16, kind="ExternalInput").ap()
    msg0hl_d = nc.dram_tensor("msg0hl", [16, BC], BF16, kind="ExternalInput").ap()
    msg1hl_d = nc.dram_tensor("msg1hl", [4, BC], BF16, kind="ExternalInput").ap()
    aux_d = nc.dram_tensor("aux", [BC // 512, 128, 288], F32, kind="ExternalInput").ap()
    w1ah_d = nc.dram_tensor("w1ah", [128, 256], BF16, kind="ExternalInput").ap()
    w1al_d = nc.dram_tensor("w1al", [128, 256], BF16, kind="ExternalInput").ap()
    w1bh_d = nc.dram_tensor("w1bh", [16, 64], BF16, kind="ExternalInput").ap()
    w1bl_d = nc.dram_tensor("w1bl", [16, 64], BF16, kind="ExternalInput").ap()
    wc_d = nc.dram_tensor("wc", [64, 8], F32, kind="ExternalInput").ap()
    wt1ah_d = nc.dram_tensor("wt1ah", [128, 64], BF16, kind="ExternalInput").ap()
    wt1al_d = nc.dram_tensor("wt1al", [128, 64], BF16, kind="ExternalInput").ap()
    wt1bh_d = nc.dram_tensor("wt1bh", [4, 64], BF16, kind="ExternalInput").ap()
    wt1bl_d = nc.dram_tensor("wt1bl", [4, 64], BF16, kind="ExternalInput").ap()
    wt2_d = nc.dram_tensor("wt2", [64, 32], F32, kind="ExternalInput").ap()
    wt_d = nc.dram_tensor("wt", [32, 64], F32, kind="ExternalInput").ap()
    b1c_d = nc.dram_tensor("b1c", [64, 1], F32, kind="ExternalInput").ap()
    bt1c_d = nc.dram_tensor("bt1c", [64, 1], F32, kind="ExternalInput").ap()
    bt2c_d = nc.dram_tensor("bt2c", [32, 1], F32, kind="ExternalInput").ap()
    bct_d = nc.dram_tensor("bctile", [128, 32], F32, kind="ExternalInput").ap()
    btt_d = nc.dram_tensor("bttile", [128, 256], F32, kind="ExternalInput").ap()
    out_d = nc.dram_tensor("out", [NG2, 128, 1184], F32, kind="ExternalOutput").ap()

    # [1024, BC] -> [p, k, g, r] with row = k*128+p, col = g*512+r
    ctxt2_v = ctxt2_d.rearrange("(k p) (g r) -> p k g r", p=128, r=512)

    with tile.TileContext(nc) as tc:
        with (
            tc.tile_pool(name="const", bufs=1) as cpool,
            tc.tile_pool(name="sb", bufs=3) as sb,
            tc.tile_pool(name="aux", bufs=8) as auxp,
            tc.tile_pool(name="stage", bufs=2) as stp,
            tc.tile_pool(name="gath", bufs=2) as gap,
            tc.tile_pool(name="msg", bufs=2) as msgp,
            tc.tile_pool(name="psx1", bufs=3, space="PSUM") as psx,
            tc.tile_pool(name="psdl", bufs=2, space="PSUM") as psd,
            tc.tile_pool(name="psix", bufs=1, space="PSUM") as psi,
            tc.tile_pool(name="psch", bufs=2, space="PSUM") as psc,
        ):
            # ---- constants ----
            id128 = cpool.tile([128, 128], F32, tag="id128")
            masks.make_identity(nc, id128[:])
            repl = cpool.tile([16, 128], F32, tag="repl")
            nc.gpsimd.memset(repl[:], 0.0)
            nc.gpsimd.affine_select(
                out=repl[:], in_=repl[:], compare_op=ALU.not_equal, fill=1.0,
                base=0, pattern=[[0, 8], [-1, 16]], channel_multiplier=1,
            )
            ir64i = cpool.tile([128, 256], I32, tag="ir64i")
            nc.gpsimd.iota(ir64i[:], pattern=[[0, 4], [-1, 64]], base=64, channel_multiplier=0)
            ir64f = cpool.tile([128, 256], F32, tag="ir64f")
            nc.vector.tensor_copy(ir64f[:], ir64i[:])
            ir8i = cpool.tile([128, 32], I32, tag="ir8i")
            nc.gpsimd.iota(ir8i[:], pattern=[[0, 4], [-1, 8]], base=8, channel_multiplier=0)
            ir8f = cpool.tile([128, 32], F32, tag="ir8f")
            nc.vector.tensor_copy(ir8f[:], ir8i[:])
            i8ji = cpool.tile([16, 128], I32, tag="i8ji")
            nc.gpsimd.iota(i8ji[:], pattern=[[8, 128]], base=0, channel_multiplier=0)
            i8jf = cpool.tile([16, 128], F32, tag="i8jf")
            nc.vector.tensor_copy(i8jf[:], i8ji[:])
            qveci = cpool.tile([16, 1], I32, tag="qveci")
            nc.gpsimd.iota(qveci[:], pattern=[[0, 1]], base=0, channel_multiplier=1024)
            qvecf = cpool.tile([16, 1], F32, tag="qvecf")
            nc.vector.tensor_copy(qvecf[:], qveci[:])

            def cload(name, shape, src, dt=F32):
                t = cpool.tile(shape, dt, tag=name)
                nc.sync.dma_start(out=t[:], in_=src)
                return t

            w1ah = cload("w1ah", [128, 4, 64], w1ah_d, BF16)
            w1al = cload("w1al", [128, 4, 64], w1al_d, BF16)
            w1bh = cload("w1bh", [16, 64], w1bh_d, BF16)
            w1bl = cload("w1bl", [16, 64], w1bl_d, BF16)
            wc = cload("wc", [64, 8], wc_d)
            wt1ah = cload("wt1ah", [128, 64], wt1ah_d, BF16)
            wt1al = cload("wt1al", [128, 64], wt1al_d, BF16)
            wt1bh = cload("wt1bh", [4, 64], wt1bh_d, BF16)
            wt1bl = cload("wt1bl", [4, 64], wt1bl_d, BF16)
            wt2 = cload("wt2", [64, 32], wt2_d)
            wt = cload("wt", [32, 64], wt_d)
            b1c = cload("b1c", [64, 1], b1c_d)
            bt1c = cload("bt1c", [64, 1], bt1c_d)
            bt2c = cload("bt2c", [32, 1], bt2c_d)
            bct = cload("bct", [128, 32], bct_d)
            btt = cload("btt", [128, 256], btt_d)

            for gg in range(NG2):
                msg0hl = msgp.tile([16, 2048], BF16, tag="msg0hl")
                nc.gpsimd.dma_start(out=msg0hl[:], in_=msg0hl_d[:, gg * 2048:(gg + 1) * 2048])
                msg1hl = msgp.tile([4, 2048], BF16, tag="msg1hl")
                nc.gpsimd.dma_start(out=msg1hl[:], in_=msg1hl_d[:, gg * 2048:(gg + 1) * 2048])
                dim_f = gap.tile([128, 16], F32, tag="dim_f")
                ostage = stp.tile([128, 1184], F32, tag="ostage")
                auxs = []

                # ---------- dim branch (natural row order) ----------
                for g4 in range(4):
                    g = gg * 4 + g4
                    ctxt_sb = sb.tile([128, 8, 512], BF16, tag="ctxt_sb")
                    nc.sync.dma_start(out=ctxt_sb[:], in_=ctxt2_v[:, :, g, :])
                    aux_sb = auxp.tile([128, 288], F32, tag="aux_sb")
                    nc.scalar.dma_start(out=aux_sb[:], in_=aux_d[g])
                    auxs.append(aux_sb)

                    # x1^T = sum_k (hi@Whi + hi@Wlo + lo@Whi) + msg0 pairs
                    x1t_ps = psx.tile([64, 512], F32, tag="x1t")
                    for k in range(4):
                        nc.tensor.matmul(x1t_ps[:], w1ah[:, k, :], ctxt_sb[:, k, :],
                                         start=(k == 0), stop=False)
                        nc.tensor.matmul(x1t_ps[:], w1ah[:, k, :], ctxt_sb[:, 4 + k, :],
                                         start=False, stop=False)
                        nc.tensor.matmul(x1t_ps[:], w1al[:, k, :], ctxt_sb[:, k, :],
                                         start=False, stop=False)
                    m0 = msg0hl[:, g4 * 512:(g4 + 1) * 512]
                    nc.tensor.matmul(x1t_ps[:], w1bh[:], m0, start=False, stop=False)
                    nc.tensor.matmul(x1t_ps[:], w1bl[:], m0, start=False, stop=True)
                    x1t = sb.tile([64, 512], F32, tag="x1t_sb")
                    nc.scalar.activation(x1t[:], x1t_ps[:], AF.Relu, bias=b1c[:])

                    dl_ps = psd.tile([128, 32], F32, tag="dl")
                    for c in range(4):
                        nc.tensor.matmul(
                            dl_ps[:, c * 8:(c + 1) * 8],
                            x1t[:, c * 128:(c + 1) * 128], wc[:],
                            start=True, stop=True,
                        )
                    zb = sb.tile([128, 32], F32, tag="zb")
                    nc.vector.tensor_tensor(zb[:], dl_ps[:], bct[:], ALU.add)
                    e8 = sb.tile([128, 32], F32, tag="e8")
                    nc.scalar.activation(e8[:], zb[:], AF.Exp)
                    z2 = sb.tile([128, 32], F32, tag="z2")
                    nc.vector.tensor_tensor(z2[:], zb[:], aux_sb[:, 0:32], ALU.add)
                    z2v = z2[:].rearrange("p (g j) -> p g j", j=8)
                    m2 = sb.tile([128, 4], F32, tag="m2")
                    nc.vector.tensor_reduce(m2[:], z2v, axis=AX.X, op=ALU.max)
                    m2b = m2[:].unsqueeze(-1).broadcast_to([128, 4, 8])
                    eq = sb.tile([128, 32], F32, tag="eq8")
                    nc.vector.tensor_tensor(eq[:].rearrange("p (g j) -> p g j", j=8), z2v, m2b, ALU.is_equal)
                    t8 = sb.tile([128, 32], F32, tag="t8")
                    nc.vector.tensor_tensor(t8[:], eq[:], ir8f[:], ALU.mult)
                    rm = sb.tile([128, 4], F32, tag="rm")
                    nc.vector.tensor_reduce(rm[:], t8[:].rearrange("p (g j) -> p g j", j=8), axis=AX.X, op=ALU.max)
                    nc.vector.tensor_scalar(
                        dim_f[:, g4 * 4:(g4 + 1) * 4], rm[:], 8.0, -1.0, ALU.subtract, ALU.mult
                    )
                    s8 = sb.tile([128, 4], F32, tag="s8")
                    nc.vector.tensor_reduce(s8[:], e8[:].rearrange("p (g j) -> p g j", j=8), axis=AX.X, op=ALU.add)
                    r8 = sb.tile([128, 4], F32, tag="r8")
                    nc.vector.reciprocal(r8[:], s8[:])
                    r8b = r8[:].unsqueeze(-1).broadcast_to([128, 4, 8])
                    nc.vector.tensor_tensor(
                        ostage[:, g4 * 32:(g4 + 1) * 32].rearrange("p (g j) -> p g j", j=8),
                        e8[:].rearrange("p (g j) -> p g j", j=8), r8b, ALU.mult,
                    )

                # ---------- gather indices + dma_gather (transposed) ----------
                dimT_ps = psi.tile([16, 128], F32, tag="idx")
                nc.tensor.transpose(dimT_ps[:], dim_f[:], id128[:])
                idxf = sb.tile([16, 128], F32, tag="idxf")
                nc.scalar.activation(idxf[:], dimT_ps[:], AF.Identity, bias=qvecf[:])
                idxf2 = sb.tile([16, 128], F32, tag="idxf2")
                nc.vector.tensor_tensor(idxf2[:], idxf[:], i8jf[:], ALU.add)
                idxr_ps = psi.tile([128, 128], F32, tag="idx")
                nc.tensor.matmul(idxr_ps[:], repl[:], idxf2[:], start=True, stop=True)
                idxs16 = sb.tile([128, 128], I16, tag="idxs16")
                nc.scalar.copy(idxs16[:], idxr_ps[:])
                # gathered, feature-major with hi/lo pairs on partitions:
                # gdimT[2c+h, D] = {hi,lo}(the_dim[perm row of D, c])
                gdimT = gap.tile([128, 2048], BF16, tag="gdimT")
                table = ctxhl_d[gg * 2048:(gg + 1) * 2048, :].rearrange("r (d e) -> (r d) e", e=128)
                if variant == "nogather":
                    nc.gpsimd.memset(gdimT[:], 0.125)
                else:
                    for h in range(2):
                        nc.gpsimd.dma_gather(
                            out_ap=gdimT[:, h * 1024:(h + 1) * 1024].rearrange("p (o n) -> p o n", o=1),
                            in_ap=table,
                            idxs_ap=idxs16[:, h * 64:(h + 1) * 64],
                            num_idxs=1024, num_idxs_reg=1024, elem_size=128,
                            transpose=True, single_packet=False,
                        )
                # dim output (int32 bits into the f32 staging tile)
                nc.vector.tensor_copy(ostage[:, 1152:1168].bitcast(I32), dim_f[:])

                # ---------- target branch (sigma row order) ----------
                for g4 in range(4):
                    aux_sb = auxs[g4]
                    gd = gdimT[:, g4 * 512:(g4 + 1) * 512]
                    x2t_ps = psc.tile([64, 512], F32, tag="chain")
                    nc.tensor.matmul(x2t_ps[:], wt1ah[:], gd, start=True, stop=False)
                    nc.tensor.matmul(x2t_ps[:], wt1al[:], gd, start=False, stop=False)
                    m1 = msg1hl[:, g4 * 512:(g4 + 1) * 512]
                    nc.tensor.matmul(x2t_ps[:], wt1bh[:], m1, start=False, stop=False)
                    nc.tensor.matmul(x2t_ps[:], wt1bl[:], m1, start=False, stop=True)
                    x2t = sb.tile([64, 512], F32, tag="x2t")
                    nc.scalar.activation(x2t[:], x2t_ps[:], AF.Relu, bias=bt1c[:])
                    x3t_ps = psc.tile([32, 512], F32, tag="chain")
                    nc.tensor.matmul(x3t_ps[:], wt2[:], x2t[:], start=True, stop=True)
                    x3t = sb.tile([32, 512], F32, tag="x3t")
                    nc.scalar.activation(x3t[:], x3t_ps[:], AF.Relu, bias=bt2c[:])
                    # target logits straight in natural orientation: per chunk
                    # tl[128, 64] = x3t_chunk.T @ Wt
                    tl_ps = psc.tile([128, 256], F32, tag="chain")
                    for c in range(4):
                        nc.tensor.matmul(
                            tl_ps[:, c * 64:(c + 1) * 64],
                            x3t[:, c * 128:(c + 1) * 128], wt[:],
                            start=True, stop=True,
                        )
                    tlb = sb.tile([128, 256], F32, tag="tlb")
                    nc.vector.tensor_tensor(tlb[:], tl_ps[:], btt[:], ALU.add)
                    e64 = sb.tile([128, 256], F32, tag="e64")
                    nc.scalar.activation(e64[:], tlb[:], AF.Exp)
                    z2t = sb.tile([128, 256], F32, tag="z2t")
                    nc.vector.tensor_tensor(z2t[:], tlb[:], aux_sb[:, 32:288], ALU.add)
                    z2tv = z2t[:].rearrange("p (g j) -> p g j", j=64)
                    m2t = sb.tile([128, 4], F32, tag="m2t")
                    nc.vector.tensor_reduce(m2t[:], z2tv, axis=AX.X, op=ALU.max)
                    m2tb = m2t[:].unsqueeze(-1).broadcast_to([128, 4, 64])
                    eqt = sb.tile([128, 256], F32, tag="eqt")
                    nc.vector.tensor_tensor(eqt[:].rearrange("p (g j) -> p g j", j=64), z2tv, m2tb, ALU.is_equal)
                    t64 = sb.tile([128, 256], F32, tag="t64")
                    nc.vector.tensor_tensor(t64[:], eqt[:], ir64f[:], ALU.mult)
                    rmt = sb.tile([128, 4], F32, tag="rmt")
                    nc.vector.tensor_reduce(rmt[:], t64[:].rearrange("p (g j) -> p g j", j=64), axis=AX.X, op=ALU.max)
                    nc.vector.tensor_scalar(
                        ostage[:, 1168 + g4 * 4:1168 + (g4 + 1) * 4].bitcast(I32),
                        rmt[:], 64.0, -1.0, ALU.subtract, ALU.mult,
                    )
                    s64 = sb.tile([128, 4], F32, tag="s64")
                    nc.vector.tensor_reduce(s64[:], e64[:].rearrange("p (g j) -> p g j", j=64), axis=AX.X, op=ALU.add)
                    r64 = sb.tile([128, 4], F32, tag="r64")
                    nc.vector.reciprocal(r64[:], s64[:])
                    r64b = r64[:].unsqueeze(-1).broadcast_to([128, 4, 64])
                    nc.vector.tensor_tensor(
                        ostage[:, 128 + g4 * 256:128 + (g4 + 1) * 256].rearrange("p (g j) -> p g j", j=64),
                        e64[:].rearrange("p (g j) -> p g j", j=64), r64b, ALU.mult,
                    )

                nc.sync.dma_start(out=out_d[gg], in_=ostage[:])

    nc.compile()
    return nc


def _split_hl(x):
    """Exact bf16 hi+lo split of an f32 array."""
    hi = x.astype(BF)
    lo = (x - hi.astype(np.float32)).astype(BF)
    return hi, lo


def _interleave_cols(hi, lo):
    """[..., n] x2 -> [..., 2n] with [..., 2c]=hi, [..., 2c+1]=lo."""
    out = np.empty(hi.shape[:-1] + (hi.shape[-1] * 2,), dtype=hi.dtype)
    out[..., 0::2] = hi
    out[..., 1::2] = lo
    return out


def _dup_rows(w):
    """[k, m] -> [2k, m] with rows duplicated pairwise."""
    return np.repeat(w, 2, axis=0)


def host_prepare(inputs, BC, n_cores=N_CORES):
    """Slice/transform full inputs into per-core in_maps."""
    import jax
    import jax.numpy as jnp

    contexts = np.ascontiguousarray(np.asarray(inputs["contexts"], dtype=np.float32))
    msg0 = np.asarray(inputs["msg0"], dtype=np.float32)
    msg1 = np.asarray(inputs["msg1"], dtype=np.float32)
    W1 = np.asarray(inputs["W1"], dtype=np.float32)
    b1 = np.asarray(inputs["b1"], dtype=np.float32)
    Wc = np.asarray(inputs["Wc"], dtype=np.float32)
    bc = np.asarray(inputs["bc"], dtype=np.float32)
    Wt1 = np.asarray(inputs["Wt1"], dtype=np.float32)
    bt1 = np.asarray(inputs["bt1"], dtype=np.float32)
    Wt2 = np.asarray(inputs["Wt2"], dtype=np.float32)
    bt2 = np.asarray(inputs["bt2"], dtype=np.float32)
    Wt = np.asarray(inputs["Wt"], dtype=np.float32)
    bt = np.asarray(inputs["bt"], dtype=np.float32)

    Btot = contexts.shape[0]
    assert Btot == BC * n_cores

    cpu = jax.devices("cpu")[0]
    with jax.default_device(cpu):
        G0 = np.asarray(jax.random.gumbel(jax.random.key(42), (Btot, NDIMS), jnp.float32))
        G1 = np.asarray(jax.random.gumbel(jax.random.key(43), (Btot, TGT), jnp.float32))

    NG2 = BC // 2048
    NG = BC // 512

    # weight preps (shared across cores)
    W1a_hi, W1a_lo = _split_hl(W1[:512])
    w1ah = np.ascontiguousarray(W1a_hi.reshape(4, 128, 64).transpose(1, 0, 2).reshape(128, 256))
    w1al = np.ascontiguousarray(W1a_lo.reshape(4, 128, 64).transpose(1, 0, 2).reshape(128, 256))
    W1b_hi, W1b_lo = _split_hl(W1[512:520])
    Wt1a_hi, Wt1a_lo = _split_hl(Wt1[:64])
    Wt1b_hi, Wt1b_lo = _split_hl(Wt1[64:66])
    shared = {
        "w1ah": w1ah, "w1al": w1al,
        "w1bh": _dup_rows(W1b_hi), "w1bl": _dup_rows(W1b_lo),
        "wc": Wc,
        "wt1ah": _dup_rows(Wt1a_hi), "wt1al": _dup_rows(Wt1a_lo),
        "wt1bh": _dup_rows(Wt1b_hi), "wt1bl": _dup_rows(Wt1b_lo),
        "wt2": Wt2, "wt": Wt,
        "b1c": b1[:, None], "bt1c": bt1[:, None], "bt2c": bt2[:, None],
        "bctile": np.ascontiguousarray(np.tile(bc, (128, 4))),
        "bttile": np.ascontiguousarray(np.tile(bt, (128, 4))),
    }

    in_maps = []
    for c in range(n_cores):
        sl = slice(c * BC, (c + 1) * BC)
        ctx_c = contexts[sl]
        hi, lo = _split_hl(ctx_c)
        ctxhl_c = np.ascontiguousarray(_interleave_cols(hi, lo))  # [BC, 1024]
        ctxt2_c = np.ascontiguousarray(
            np.concatenate([hi.T, lo.T], axis=0)
        )  # [1024, BC]
        m0h, m0l = _split_hl(msg0[sl])
        msg0hl_c = np.ascontiguousarray(_interleave_cols(m0h, m0l).T)  # [16, BC]
        msg1s = msg1[sl].reshape(NG2, 2048, 2)[:, PERM2048, :].reshape(BC, 2)
        m1h, m1l = _split_hl(msg1s)
        msg1hl_c = np.ascontiguousarray(_interleave_cols(m1h, m1l).T)  # [4, BC]
        g0t = np.ascontiguousarray(
            G0[sl].reshape(NG, 4, 128, 8).transpose(0, 2, 1, 3).reshape(NG, 128, 32)
        )
        g1s = G1[sl].reshape(NG2, 2048, 64)[:, PERM2048, :]
        g1t = g1s.reshape(NG2, 4, 4, 128, 64).transpose(0, 1, 3, 2, 4).reshape(NG, 128, 256)
        aux_c = np.ascontiguousarray(np.concatenate([g0t, g1t], axis=2))
        in_maps.append(
            dict(
                ctxhl=ctxhl_c, ctxt2=ctxt2_c, msg0hl=msg0hl_c, msg1hl=msg1hl_c,
                aux=aux_c, **shared,
            )
        )
    return in_maps


def host_unpack(results, BC, n_cores=N_CORES):
    """Reassemble per-core 'out' arrays into the full reference output tuple."""
    NG2 = BC // 2048
    Btot = BC * n_cores
    dim_probs = np.empty((Btot, NDIMS), np.float32)
    target_probs = np.empty((Btot, TGT), np.float32)
    dim = np.empty((Btot,), np.int32)
    target = np.empty((Btot,), np.int32)
    for c in range(n_cores):
        o = results[c]["out"]  # [NG2, 128, 1184] f32
        sl = slice(c * BC, (c + 1) * BC)
        dp = o[:, :, 0:128].reshape(NG2, 128, 4, 4, 8).transpose(0, 2, 3, 1, 4)
        dim_probs[sl] = dp.reshape(BC, NDIMS)
        tp_dev = o[:, :, 128:1152].reshape(NG2, 128, 4, 4, 64).transpose(0, 2, 3, 1, 4)
        tp_dev = tp_dev.reshape(NG2, 2048, 64)
        tp = np.empty_like(tp_dev)
        tp[:, PERM2048, :] = tp_dev
        target_probs[sl] = tp.reshape(BC, TGT)
        dim_dev = np.ascontiguousarray(o[:, :, 1152:1168]).view(np.int32)
        dim[sl] = dim_dev.transpose(0, 2, 1).reshape(BC)
        tg_dev = np.ascontiguousarray(o[:, :, 1168:1184]).view(np.int32)
        tg_dev = tg_dev.transpose(0, 2, 1).reshape(NG2, 2048)
        tg = np.empty_like(tg_dev)
        tg[:, PERM2048] = tg_dev
        target[sl] = tg.reshape(BC)
    return dim_probs, target_probs, dim, target


_CACHE = {}


def _get_program(BC, n_cores):
    key = (BC, n_cores)
    if key not in _CACHE:
        _CACHE[key] = build_program(BC, n_cores)
    return _CACHE[key]


def run(inputs, BC=B // N_CORES, n_cores=N_CORES, trace=False, **kw):
    nc = _get_program(BC, n_cores)
    in_maps = host_prepare(inputs, BC, n_cores)
    res = run_bass_kernel_spmd(nc, in_maps, core_ids=list(range(n_cores)), trace=trace, **kw)
    return host_unpack(res.results, BC, n_cores), res


def kernel(**inputs):
    (dim_probs, target_probs, dim, target), _ = run(inputs)
    return dim_probs, target_probs, dim, target


# revision 9
# speedup vs baseline: 1.7223x; 1.3235x over previous
"""Trainium2 Bass kernel for nn_DimReceiver (moe_routing), pure data-parallel
over 8 NeuronCores.

Math (per row):
  x1         = relu([ctx, msg0] @ W1 + b1)                  # [520] -> [64]
  dim_logits = x1 @ Wc + bc                                 # [8]
  dim        = argmax(dim_logits + G0)   (G0 = Gumbel noise of key(42), host-made)
  dim_probs  = softmax(dim_logits)
  the_dim    = ctx[dim*64 : dim*64+64]                      # per-row gather
  x2         = relu([the_dim, msg1] @ Wt1 + bt1)
  x3         = relu(x2 @ Wt2 + bt2)
  t_logits   = x3 @ Wt + bt                                 # [64]
  target     = argmax(t_logits + G1)     (G1 = Gumbel noise of key(43))
  target_probs = softmax(t_logits)

Device layout / numerics notes:
  - Matmuls run feature-major (features on partitions, rows on the free axis).
    fp32 matmuls on TRN2 run 5x slower (fp32_mode=LOW_HIGH), so the big
    contractions use exact bf16 hi+lo splits prepared on host:
      x @ W = hi@Whi + hi@Wlo + lo@Whi  (+ lo@Wlo ~ 2^-18, dropped)
    which is fp32-accurate at bf16 matmul speed.
  - the_dim is fetched with dma_gather(transpose=True) from a host-built
    interleaved table ctx_hl[row, dim] = [c0hi, c0lo, c1hi, c1lo, ...] so the
    gathered tile lands feature-major with hi/lo pairs on partitions; weights
    with duplicated rows (W'[2c]=W'[2c+1]=W[c]) consume it directly:
      x2 = gdimT@Whi_dup + gdimT@Wlo_dup  (exact product of (hi+lo)(Whi+Wlo))
  - One dma_gather is capped at ~1024 idxs (128 SWDGE ring descriptors per Q7
    core); two half-gathers per 2048-row block preserve the position mapping.
  - The gather's HW layout imposes a fixed row permutation sigma within each
    2048-row block; the target branch runs in sigma order and the host
    un-permutes its outputs.
  - Gumbel noise is precomputed on host with jax (CPU) so sampling matches
    jax.random.categorical bit-for-bit up to matmul rounding.
"""

import sys

sys.path.insert(0, "/opt/trn_rl_repo")

import ml_dtypes
import numpy as np

import concourse.bacc as bacc
import concourse.bass as bass
from concourse import masks, mybir, tile
from concourse.bass_utils import run_bass_kernel_spmd

F32 = mybir.dt.float32
BF16 = mybir.dt.bfloat16
I32 = mybir.dt.int32
I16 = mybir.dt.int16
AF = mybir.ActivationFunctionType
ALU = mybir.AluOpType
AX = mybir.AxisListType
BF = np.dtype(ml_dtypes.bfloat16)

B = 262144
CTX = 512
NDIMS = 8
TGT = 64
N_CORES = 8

# per-2048 gather row permutation: device position i holds source row perm[i]
PERM2048 = (np.arange(2048) % 16) * 128 + (np.arange(2048) // 16)


def build_program(BC, n_cores=N_CORES, debug=False, enable_asserts=False, variant="full"):
    """Build + compile the Bass/Tile program for BC rows per core."""
    assert BC % 2048 == 0
    NG2 = BC // 2048  # gather groups
    nc = bacc.Bacc(
        "TRN2",
        target_bir_lowering=False,
        debug=debug,
        enable_asserts=enable_asserts,
        num_devices=n_cores,
    )

    # gather table: [BC, 1024] bf16, row r = [c0hi c0lo c1hi c1lo ...]
    ctxhl_d = nc.dram_tensor("ctxhl", [BC, 1024], BF16, kind="ExternalInput").ap()
    # x1 streams: rows 0..511 = ctxT_hi k-tiles, 512..1023 = ctxT_lo
    ctxt2_d = nc.dram_tensor("ctxt2", [1024, BC], BF16, kind="ExternalInput").ap()
    msg0hl_d = nc.dram_tensor("msg0hl", [16, BC], BF16, kind="ExternalInput").ap()
    msg1hl_d = nc.dram_tensor("msg1hl", [4, BC], BF16, kind="ExternalInput").ap()
    aux_d = nc.dram_tensor("aux", [BC // 512, 128, 288], F32, kind="ExternalInput").ap()
    w1ah_d = nc.dram_tensor("w1ah", [128, 256], BF16, kind="ExternalInput").ap()
    w1al_d = nc.dram_tensor("w1al", [128, 256], BF16, kind="ExternalInput").ap()
    w1bh_d = nc.dram_tensor("w1bh", [16, 64], BF16, kind="ExternalInput").ap()
    w1bl_d = nc.dram_tensor("w1bl", [16, 64], BF16, kind="ExternalInput").ap()
    wc_d = nc.dram_tensor("wc", [64, 8], F32, kind="ExternalInput").ap()
    wt1ah_d = nc.dram_tensor("wt1ah", [128, 64], BF16, kind="ExternalInput").ap()
    wt1al_d = nc.dram_tensor("wt1al", [128, 64], BF16, kind="ExternalInput").ap()
    wt1bh_d = nc.dram_tensor("wt1bh", [4, 64], BF16, kind="ExternalInput").ap()
    wt1bl_d = nc.dram_tensor("wt1bl", [4, 64], BF16, kind="ExternalInput").ap()
    wt2_d = nc.dram_tensor("wt2", [64, 32], F32, kind="ExternalInput").ap()
    wt_d = nc.dram_tensor("wt", [32, 64], F32, kind="ExternalInput").ap()
    b1c_d = nc.dram_tensor("b1c", [64, 1], F32, kind="ExternalInput").ap()
    bt1c_d = nc.dram_tensor("bt1c", [64, 1], F32, kind="ExternalInput").ap()
    bt2c_d = nc.dram_tensor("bt2c", [32, 1], F32, kind="ExternalInput").ap()
    bct_d = nc.dram_tensor("bctile", [128, 32], F32, kind="ExternalInput").ap()
    btt_d = nc.dram_tensor("bttile", [128, 256], F32, kind="ExternalInput").ap()
    out_d = nc.dram_tensor("out", [NG2, 128, 1184], F32, kind="ExternalOutput").ap()

    # [1024, BC] -> [p, k, g, r] with row = k*128+p, col = g*512+r
    ctxt2_v = ctxt2_d.rearrange("(k p) (g r) -> p k g r", p=128, r=512)

    with tile.TileContext(nc) as tc:
        with (
            tc.tile_pool(name="const", bufs=1) as cpool,
            tc.tile_pool(name="sb", bufs=3) as sb,
            tc.tile_pool(name="aux", bufs=12) as auxp,
            tc.tile_pool(name="stage", bufs=3) as stp,
            tc.tile_pool(name="gath", bufs=3) as gap,
            tc.tile_pool(name="msg", bufs=3) as msgp,
            tc.tile_pool(name="psx1", bufs=3, space="PSUM") as psx,
            tc.tile_pool(name="psdl", bufs=2, space="PSUM") as psd,
            tc.tile_pool(name="psix", bufs=1, space="PSUM") as psi,
            tc.tile_pool(name="psch", bufs=2, space="PSUM") as psc,
        ):
            # ---- constants ----
            id128 = cpool.tile([128, 128], F32, tag="id128")
            masks.make_identity(nc, id128[:])
            repl = cpool.tile([16, 128], F32, tag="repl")
            nc.gpsimd.memset(repl[:], 0.0)
            nc.gpsimd.affine_select(
                out=repl[:], in_=repl[:], compare_op=ALU.not_equal, fill=1.0,
                base=0, pattern=[[0, 8], [-1, 16]], channel_multiplier=1,
            )
            ir64i = cpool.tile([128, 256], I32, tag="ir64i")
            nc.gpsimd.iota(ir64i[:], pattern=[[0, 4], [-1, 64]], base=64, channel_multiplier=0)
            ir64f = cpool.tile([128, 256], F32, tag="ir64f")
            nc.vector.tensor_copy(ir64f[:], ir64i[:])
            ir8i = cpool.tile([128, 32], I32, tag="ir8i")
            nc.gpsimd.iota(ir8i[:], pattern=[[0, 4], [-1, 8]], base=8, channel_multiplier=0)
            ir8f = cpool.tile([128, 32], F32, tag="ir8f")
            nc.vector.tensor_copy(ir8f[:], ir8i[:])
            i8ji = cpool.tile([16, 128], I32, tag="i8ji")
            nc.gpsimd.iota(i8ji[:], pattern=[[8, 128]], base=0, channel_multiplier=0)
            i8jf = cpool.tile([16, 128], F32, tag="i8jf")
            nc.vector.tensor_copy(i8jf[:], i8ji[:])
            qveci = cpool.tile([16, 1], I32, tag="qveci")
            nc.gpsimd.iota(qveci[:], pattern=[[0, 1]], base=0, channel_multiplier=1024)
            qvecf = cpool.tile([16, 1], F32, tag="qvecf")
            nc.vector.tensor_copy(qvecf[:], qveci[:])

            def cload(name, shape, src, dt=F32):
                t = cpool.tile(shape, dt, tag=name)
                nc.sync.dma_start(out=t[:], in_=src)
                return t

            w1ah = cload("w1ah", [128, 4, 64], w1ah_d, BF16)
            w1al = cload("w1al", [128, 4, 64], w1al_d, BF16)
            w1bh = cload("w1bh", [16, 64], w1bh_d, BF16)
            w1bl = cload("w1bl", [16, 64], w1bl_d, BF16)
            wc = cload("wc", [64, 8], wc_d)
            wt1ah = cload("wt1ah", [128, 64], wt1ah_d, BF16)
            wt1al = cload("wt1al", [128, 64], wt1al_d, BF16)
            wt1bh = cload("wt1bh", [4, 64], wt1bh_d, BF16)
            wt1bl = cload("wt1bl", [4, 64], wt1bl_d, BF16)
            wt2 = cload("wt2", [64, 32], wt2_d)
            wt = cload("wt", [32, 64], wt_d)
            b1c = cload("b1c", [64, 1], b1c_d)
            bt1c = cload("bt1c", [64, 1], bt1c_d)
            bt2c = cload("bt2c", [32, 1], bt2c_d)
            bct = cload("bct", [128, 32], bct_d)
            btt = cload("btt", [128, 256], btt_d)

            state = {}

            def dim_and_gather(gg):
                msg0hl = msgp.tile([16, 2048], BF16, tag="msg0hl")
                nc.gpsimd.dma_start(out=msg0hl[:], in_=msg0hl_d[:, gg * 2048:(gg + 1) * 2048])
                msg1hl = msgp.tile([4, 2048], BF16, tag="msg1hl")
                nc.gpsimd.dma_start(out=msg1hl[:], in_=msg1hl_d[:, gg * 2048:(gg + 1) * 2048])
                dim_f = gap.tile([128, 16], F32, tag="dim_f")
                ostage = stp.tile([128, 1184], F32, tag="ostage")
                auxs = []

                # ---------- dim branch (natural row order) ----------
                for g4 in range(4):
                    g = gg * 4 + g4
                    ctxt_sb = sb.tile([128, 8, 512], BF16, tag="ctxt_sb")
                    nc.sync.dma_start(out=ctxt_sb[:], in_=ctxt2_v[:, :, g, :])
                    aux_sb = auxp.tile([128, 288], F32, tag="aux_sb")
                    nc.scalar.dma_start(out=aux_sb[:], in_=aux_d[g])
                    auxs.append(aux_sb)

                    # x1^T = sum_k (hi@Whi + hi@Wlo + lo@Whi) + msg0 pairs
                    x1t_ps = psx.tile([64, 512], F32, tag="x1t")
                    for k in range(4):
                        nc.tensor.matmul(x1t_ps[:], w1ah[:, k, :], ctxt_sb[:, k, :],
                                         start=(k == 0), stop=False)
                        nc.tensor.matmul(x1t_ps[:], w1ah[:, k, :], ctxt_sb[:, 4 + k, :],
                                         start=False, stop=False)
                        nc.tensor.matmul(x1t_ps[:], w1al[:, k, :], ctxt_sb[:, k, :],
                                         start=False, stop=False)
                    m0 = msg0hl[:, g4 * 512:(g4 + 1) * 512]
                    nc.tensor.matmul(x1t_ps[:], w1bh[:], m0, start=False, stop=False)
                    nc.tensor.matmul(x1t_ps[:], w1bl[:], m0, start=False, stop=True)
                    x1t = sb.tile([64, 512], F32, tag="x1t_sb")
                    nc.scalar.activation(x1t[:], x1t_ps[:], AF.Relu, bias=b1c[:])

                    dl_ps = psd.tile([128, 32], F32, tag="dl")
                    for c in range(4):
                        nc.tensor.matmul(
                            dl_ps[:, c * 8:(c + 1) * 8],
                            x1t[:, c * 128:(c + 1) * 128], wc[:],
                            start=True, stop=True,
                        )
                    zb = sb.tile([128, 32], F32, tag="zb")
                    nc.vector.tensor_tensor(zb[:], dl_ps[:], bct[:], ALU.add)
                    e8 = sb.tile([128, 32], F32, tag="e8")
                    nc.scalar.activation(e8[:], zb[:], AF.Exp)
                    z2 = sb.tile([128, 32], F32, tag="z2")
                    nc.vector.tensor_tensor(z2[:], zb[:], aux_sb[:, 0:32], ALU.add)
                    z2v = z2[:].rearrange("p (g j) -> p g j", j=8)
                    m2 = sb.tile([128, 4], F32, tag="m2")
                    nc.vector.tensor_reduce(m2[:], z2v, axis=AX.X, op=ALU.max)
                    m2b = m2[:].unsqueeze(-1).broadcast_to([128, 4, 8])
                    eq = sb.tile([128, 32], F32, tag="eq8")
                    nc.vector.tensor_tensor(eq[:].rearrange("p (g j) -> p g j", j=8), z2v, m2b, ALU.is_equal)
                    t8 = sb.tile([128, 32], F32, tag="t8")
                    nc.vector.tensor_tensor(t8[:], eq[:], ir8f[:], ALU.mult)
                    rm = sb.tile([128, 4], F32, tag="rm")
                    nc.vector.tensor_reduce(rm[:], t8[:].rearrange("p (g j) -> p g j", j=8), axis=AX.X, op=ALU.max)
                    nc.vector.tensor_scalar(
                        dim_f[:, g4 * 4:(g4 + 1) * 4], rm[:], 8.0, -1.0, ALU.subtract, ALU.mult
                    )
                    s8 = sb.tile([128, 4], F32, tag="s8")
                    nc.vector.tensor_reduce(s8[:], e8[:].rearrange("p (g j) -> p g j", j=8), axis=AX.X, op=ALU.add)
                    r8 = sb.tile([128, 4], F32, tag="r8")
                    nc.vector.reciprocal(r8[:], s8[:])
                    r8b = r8[:].unsqueeze(-1).broadcast_to([128, 4, 8])
                    nc.vector.tensor_tensor(
                        ostage[:, g4 * 32:(g4 + 1) * 32].rearrange("p (g j) -> p g j", j=8),
                        e8[:].rearrange("p (g j) -> p g j", j=8), r8b, ALU.mult,
                    )

                # ---------- gather indices + dma_gather (transposed) ----------
                dimT_ps = psi.tile([16, 128], F32, tag="idx")
                nc.tensor.transpose(dimT_ps[:], dim_f[:], id128[:])
                idxf = sb.tile([16, 128], F32, tag="idxf")
                nc.scalar.activation(idxf[:], dimT_ps[:], AF.Identity, bias=qvecf[:])
                idxf2 = sb.tile([16, 128], F32, tag="idxf2")
                nc.vector.tensor_tensor(idxf2[:], idxf[:], i8jf[:], ALU.add)
                idxr_ps = psi.tile([128, 128], F32, tag="idx")
                nc.tensor.matmul(idxr_ps[:], repl[:], idxf2[:], start=True, stop=True)
                idxs16 = sb.tile([128, 128], I16, tag="idxs16")
                nc.scalar.copy(idxs16[:], idxr_ps[:])
                # gathered, feature-major with hi/lo pairs on partitions:
                # gdimT[2c+h, D] = {hi,lo}(the_dim[perm row of D, c])
                gdimT = gap.tile([128, 2048], BF16, tag="gdimT")
                table = ctxhl_d[gg * 2048:(gg + 1) * 2048, :].rearrange("r (d e) -> (r d) e", e=128)
                if variant == "nogather":
                    nc.gpsimd.memset(gdimT[:], 0.125)
                else:
                    for h in range(2):
                        nc.gpsimd.dma_gather(
                            out_ap=gdimT[:, h * 1024:(h + 1) * 1024].rearrange("p (o n) -> p o n", o=1),
                            in_ap=table,
                            idxs_ap=idxs16[:, h * 64:(h + 1) * 64],
                            num_idxs=1024, num_idxs_reg=1024, elem_size=128,
                            transpose=True, single_packet=False,
                        )
                # dim output (int32 bits into the f32 staging tile)
                nc.vector.tensor_copy(ostage[:, 1152:1168].bitcast(I32), dim_f[:])

                state[gg] = dict(ostage=ostage, auxs=auxs, msg1hl=msg1hl, gdimT=gdimT)

            def target_branch(gg):
                st = state.pop(gg)
                ostage, auxs, msg1hl, gdimT = st["ostage"], st["auxs"], st["msg1hl"], st["gdimT"]
                # ---------- target branch (sigma row order) ----------
                for g4 in range(4):
                    aux_sb = auxs[g4]
                    gd = gdimT[:, g4 * 512:(g4 + 1) * 512]
                    x2t_ps = psc.tile([64, 512], F32, tag="chain")
                    nc.tensor.matmul(x2t_ps[:], wt1ah[:], gd, start=True, stop=False)
                    nc.tensor.matmul(x2t_ps[:], wt1al[:], gd, start=False, stop=False)
                    m1 = msg1hl[:, g4 * 512:(g4 + 1) * 512]
                    nc.tensor.matmul(x2t_ps[:], wt1bh[:], m1, start=False, stop=False)
                    nc.tensor.matmul(x2t_ps[:], wt1bl[:], m1, start=False, stop=True)
                    x2t = sb.tile([64, 512], F32, tag="x2t")
                    nc.scalar.activation(x2t[:], x2t_ps[:], AF.Relu, bias=bt1c[:])
                    x3t_ps = psc.tile([32, 512], F32, tag="chain")
                    nc.tensor.matmul(x3t_ps[:], wt2[:], x2t[:], start=True, stop=True)
                    x3t = sb.tile([32, 512], F32, tag="x3t")
                    nc.scalar.activation(x3t[:], x3t_ps[:], AF.Relu, bias=bt2c[:])
                    # target logits straight in natural orientation: per chunk
                    # tl[128, 64] = x3t_chunk.T @ Wt
                    tl_ps = psc.tile([128, 256], F32, tag="chain")
                    for c in range(4):
                        nc.tensor.matmul(
                            tl_ps[:, c * 64:(c + 1) * 64],
                            x3t[:, c * 128:(c + 1) * 128], wt[:],
                            start=True, stop=True,
                        )
                    tlb = sb.tile([128, 256], F32, tag="tlb")
                    nc.vector.tensor_tensor(tlb[:], tl_ps[:], btt[:], ALU.add)
                    e64 = sb.tile([128, 256], F32, tag="e64")
                    nc.scalar.activation(e64[:], tlb[:], AF.Exp)
                    z2t = sb.tile([128, 256], F32, tag="z2t")
                    nc.vector.tensor_tensor(z2t[:], tlb[:], aux_sb[:, 32:288], ALU.add)
                    z2tv = z2t[:].rearrange("p (g j) -> p g j", j=64)
                    m2t = sb.tile([128, 4], F32, tag="m2t")
                    nc.vector.tensor_reduce(m2t[:], z2tv, axis=AX.X, op=ALU.max)
                    m2tb = m2t[:].unsqueeze(-1).broadcast_to([128, 4, 64])
                    eqt = sb.tile([128, 256], F32, tag="eqt")
                    nc.vector.tensor_tensor(eqt[:].rearrange("p (g j) -> p g j", j=64), z2tv, m2tb, ALU.is_equal)
                    t64 = sb.tile([128, 256], F32, tag="t64")
                    nc.vector.tensor_tensor(t64[:], eqt[:], ir64f[:], ALU.mult)
                    rmt = sb.tile([128, 4], F32, tag="rmt")
                    nc.vector.tensor_reduce(rmt[:], t64[:].rearrange("p (g j) -> p g j", j=64), axis=AX.X, op=ALU.max)
                    nc.vector.tensor_scalar(
                        ostage[:, 1168 + g4 * 4:1168 + (g4 + 1) * 4].bitcast(I32),
                        rmt[:], 64.0, -1.0, ALU.subtract, ALU.mult,
                    )
                    s64 = sb.tile([128, 4], F32, tag="s64")
                    nc.vector.tensor_reduce(s64[:], e64[:].rearrange("p (g j) -> p g j", j=64), axis=AX.X, op=ALU.add)
                    r64 = sb.tile([128, 4], F32, tag="r64")
                    nc.vector.reciprocal(r64[:], s64[:])
                    r64b = r64[:].unsqueeze(-1).broadcast_to([128, 4, 64])
                    nc.vector.tensor_tensor(
                        ostage[:, 128 + g4 * 256:128 + (g4 + 1) * 256].rearrange("p (g j) -> p g j", j=64),
                        e64[:].rearrange("p (g j) -> p g j", j=64), r64b, ALU.mult,
                    )

                nc.sync.dma_start(out=out_d[gg], in_=ostage[:])

            # software pipeline: overlap gather/target of gg with dim of gg+1
            for stage in range(NG2 + 1):
                if stage < NG2:
                    dim_and_gather(stage)
                if stage >= 1:
                    target_branch(stage - 1)

    nc.compile()
    return nc


def _split_hl(x):
    """Exact bf16 hi+lo split of an f32 array."""
    hi = x.astype(BF)
    lo = (x - hi.astype(np.float32)).astype(BF)
    return hi, lo


def _interleave_cols(hi, lo):
    """[..., n] x2 -> [..., 2n] with [..., 2c]=hi, [..., 2c+1]=lo."""
    out = np.empty(hi.shape[:-1] + (hi.shape[-1] * 2,), dtype=hi.dtype)
    out[..., 0::2] = hi
    out[..., 1::2] = lo
    return out


def _dup_rows(w):
    """[k, m] -> [2k, m] with rows duplicated pairwise."""
    return np.repeat(w, 2, axis=0)


def host_prepare(inputs, BC, n_cores=N_CORES):
    """Slice/transform full inputs into per-core in_maps."""
    import jax
    import jax.numpy as jnp

    contexts = np.ascontiguousarray(np.asarray(inputs["contexts"], dtype=np.float32))
    msg0 = np.asarray(inputs["msg0"], dtype=np.float32)
    msg1 = np.asarray(inputs["msg1"], dtype=np.float32)
    W1 = np.asarray(inputs["W1"], dtype=np.float32)
    b1 = np.asarray(inputs["b1"], dtype=np.float32)
    Wc = np.asarray(inputs["Wc"], dtype=np.float32)
    bc = np.asarray(inputs["bc"], dtype=np.float32)
    Wt1 = np.asarray(inputs["Wt1"], dtype=np.float32)
    bt1 = np.asarray(inputs["bt1"], dtype=np.float32)
    Wt2 = np.asarray(inputs["Wt2"], dtype=np.float32)
    bt2 = np.asarray(inputs["bt2"], dtype=np.float32)
    Wt = np.asarray(inputs["Wt"], dtype=np.float32)
    bt = np.asarray(inputs["bt"], dtype=np.float32)

    Btot = contexts.shape[0]
    assert Btot == BC * n_cores

    cpu = jax.devices("cpu")[0]
    with jax.default_device(cpu):
        G0 = np.asarray(jax.random.gumbel(jax.random.key(42), (Btot, NDIMS), jnp.float32))
        G1 = np.asarray(jax.random.gumbel(jax.random.key(43), (Btot, TGT), jnp.float32))

    NG2 = BC // 2048
    NG = BC // 512

    # weight preps (shared across cores)
    W1a_hi, W1a_lo = _split_hl(W1[:512])
    w1ah = np.ascontiguousarray(W1a_hi.reshape(4, 128, 64).transpose(1, 0, 2).reshape(128, 256))
    w1al = np.ascontiguousarray(W1a_lo.reshape(4, 128, 64).transpose(1, 0, 2).reshape(128, 256))
    W1b_hi, W1b_lo = _split_hl(W1[512:520])
    Wt1a_hi, Wt1a_lo = _split_hl(Wt1[:64])
    Wt1b_hi, Wt1b_lo = _split_hl(Wt1[64:66])
    shared = {
        "w1ah": w1ah, "w1al": w1al,
        "w1bh": _dup_rows(W1b_hi), "w1bl": _dup_rows(W1b_lo),
        "wc": Wc,
        "wt1ah": _dup_rows(Wt1a_hi), "wt1al": _dup_rows(Wt1a_lo),
        "wt1bh": _dup_rows(Wt1b_hi), "wt1bl": _dup_rows(Wt1b_lo),
        "wt2": Wt2, "wt": Wt,
        "b1c": b1[:, None], "bt1c": bt1[:, None], "bt2c": bt2[:, None],
        "bctile": np.ascontiguousarray(np.tile(bc, (128, 4))),
        "bttile": np.ascontiguousarray(np.tile(bt, (128, 4))),
    }

    in_maps = []
    for c in range(n_cores):
        sl = slice(c * BC, (c + 1) * BC)
        ctx_c = contexts[sl]
        hi, lo = _split_hl(ctx_c)
        ctxhl_c = np.ascontiguousarray(_interleave_cols(hi, lo))  # [BC, 1024]
        ctxt2_c = np.ascontiguousarray(
            np.concatenate([hi.T, lo.T], axis=0)
        )  # [1024, BC]
        m0h, m0l = _split_hl(msg0[sl])
        msg0hl_c = np.ascontiguousarray(_interleave_cols(m0h, m0l).T)  # [16, BC]
        msg1s = msg1[sl].reshape(NG2, 2048, 2)[:, PERM2048, :].reshape(BC, 2)
        m1h, m1l = _split_hl(msg1s)
        msg1hl_c = np.ascontiguousarray(_interleave_cols(m1h, m1l).T)  # [4, BC]
        g0t = np.ascontiguousarray(
            G0[sl].reshape(NG, 4, 128, 8).transpose(0, 2, 1, 3).reshape(NG, 128, 32)
        )
        g1s = G1[sl].reshape(NG2, 2048, 64)[:, PERM2048, :]
        g1t = g1s.reshape(NG2, 4, 4, 128, 64).transpose(0, 1, 3, 2, 4).reshape(NG, 128, 256)
        aux_c = np.ascontiguousarray(np.concatenate([g0t, g1t], axis=2))
        in_maps.append(
            dict(
                ctxhl=ctxhl_c, ctxt2=ctxt2_c, msg0hl=msg0hl_c, msg1hl=msg1hl_c,
                aux=aux_c, **shared,
            )
        )
    return in_maps


def host_unpack(results, BC, n_cores=N_CORES):
    """Reassemble per-core 'out' arrays into the full reference output tuple."""
    NG2 = BC // 2048
    Btot = BC * n_cores
    dim_probs = np.empty((Btot, NDIMS), np.float32)
    target_probs = np.empty((Btot, TGT), np.float32)
    dim = np.empty((Btot,), np.int32)
    target = np.empty((Btot,), np.int32)
    for c in range(n_cores):
        o = results[c]["out"]  # [NG2, 128, 1184] f32
        sl = slice(c * BC, (c + 1) * BC)
        dp = o[:, :, 0:128].reshape(NG2, 128, 4, 4, 8).transpose(0, 2, 3, 1, 4)
        dim_probs[sl] = dp.reshape(BC, NDIMS)
        tp_dev = o[:, :, 128:1152].reshape(NG2, 128, 4, 4, 64).transpose(0, 2, 3, 1, 4)
        tp_dev = tp_dev.reshape(NG2, 2048, 64)
        tp = np.empty_like(tp_dev)
        tp[:, PERM2048, :] = tp_dev
        target_probs[sl] = tp.reshape(BC, TGT)
        dim_dev = np.ascontiguousarray(o[:, :, 1152:1168]).view(np.int32)
        dim[sl] = dim_dev.transpose(0, 2, 1).reshape(BC)
        tg_dev = np.ascontiguousarray(o[:, :, 1168:1184]).view(np.int32)
        tg_dev = tg_dev.transpose(0, 2, 1).reshape(NG2, 2048)
        tg = np.empty_like(tg_dev)
        tg[:, PERM2048] = tg_dev
        target[sl] = tg.reshape(BC)
    return dim_probs, target_probs, dim, target


_CACHE = {}


def _get_program(BC, n_cores):
    key = (BC, n_cores)
    if key not in _CACHE:
        _CACHE[key] = build_program(BC, n_cores)
    return _CACHE[key]


def run(inputs, BC=B // N_CORES, n_cores=N_CORES, trace=False, **kw):
    nc = _get_program(BC, n_cores)
    in_maps = host_prepare(inputs, BC, n_cores)
    res = run_bass_kernel_spmd(nc, in_maps, core_ids=list(range(n_cores)), trace=trace, **kw)
    return host_unpack(res.results, BC, n_cores), res


def kernel(**inputs):
    (dim_probs, target_probs, dim, target), _ = run(inputs)
    return dim_probs, target_probs, dim, target
